# revision 11
# baseline (speedup 1.0000x reference)
"""3-branch GCN (DGL GraphConv x3 + max-pool + MLP head) on 8 TRN2 NeuronCores.

Sharding: destination nodes (2500/core). L1's x[src] gather is a static
permutation of the input, so it is pre-gathered (and rsqrt(outdeg)-prescaled)
on the host and streamed with dense DMAs. L2/L3 gather h[src] rows from a
replicated DRAM table with batched SWDGE dma_gathers (1024 rows each, round-
robined over the 4 SWDGE queues so all four DSP pairs generate descriptors
concurrently), aggregate via count-matrix fp16 matmuls into PSUM (per-tile
unique-src dedup folds edge multiplicity into S, built host-side and streamed
from DRAM), and apply the dense W matmul per dst tile. Layer outputs are
AllGathered; layers run graph-interleaved (layer-major) so each AllGather
hides under the other two graphs' compute. Max-pool is local + a final
AllReduce(max); the tiny MLP head runs replicated on every core.
"""
import numpy as np
import concourse.bass as bass
import concourse.bacc as bacc
import concourse.tile as tile
import concourse.mybir as mybir
from concourse import library_config
from concourse.bass_utils import run_bass_kernel_spmd

NC_ = 8
N = 20000
E = 320000
SH = N // NC_          # 2500 nodes per core
NT = 20                # dst tiles per core (19 full + 68-node partial)
D_IN, D_H = 128, 304
DPAD = 384             # fp16 row pad for 768B (256B-mult) rows
GQ = 8                 # chunks per dma_gather (1024 rows = SWDGE ring size)
f16, f32 = mybir.dt.float16, mybir.dt.float32
i16 = mybir.dt.int16
AF = mybir.ActivationFunctionType
core_ids = list(range(NC_))


def _prep_graph(src, dst, x):
    """Per-core deduped edge metadata, host-built S, pre-gathered L1 operand."""
    src = np.asarray(src).astype(np.int64)
    dst = np.asarray(dst).astype(np.int64)
    outdeg = np.bincount(src, minlength=N).clip(1).astype(np.float32)
    indeg = np.bincount(dst, minlength=N).clip(1).astype(np.float32)
    rso = (1.0 / np.sqrt(outdeg)).astype(np.float32)
    rsi = (1.0 / np.sqrt(indeg)).astype(np.float32)
    xs = (np.asarray(x, np.float32) * rso[:, None]).astype(np.float16)  # [N,128]
    # per (core, tile): unique srcs + count matrix columns
    uniqs = [[None] * NT for _ in range(NC_)]
    cnts = [[None] * NT for _ in range(NC_)]
    for c in range(NC_):
        m = (dst // SH) == c
        es, ed = src[m], dst[m] - c * SH
        for t in range(NT):
            tm = (ed // 128) == t
            u, inv = np.unique(es[tm], return_inverse=True)
            cm = np.zeros((max(len(u), 1), 128), np.float16)
            np.add.at(cm, (inv, ed[tm] - t * 128), 1.0)
            uniqs[c][t] = u if len(u) else np.zeros(1, np.int64)
            cnts[c][t] = cm
    Ck = [max(int(np.ceil(len(uniqs[c][t]) / 128)) for c in range(NC_)) or 1
          for t in range(NT)]
    nch = sum(Ck)
    S = np.zeros((NC_, 128, nch, 128), np.float16)   # [slot, chunk, dstrow]
    idx16 = np.zeros((NC_, 128, nch * 8), np.int16)
    xg = np.zeros((NC_, 128, nch * 128), np.float16)
    for c in range(NC_):
        j0 = 0
        for t in range(NT):
            u, cm = uniqs[c][t], cnts[c][t]
            npad = Ck[t] * 128
            up = np.zeros(npad, np.int64)
            up[:len(u)] = u
            cp = np.zeros((npad, 128), np.float16)
            cp[:len(u)] = cm
            # slot-major: slot i of chunk k = up[k*128 + i%... linear i = k*128+p
            S[c, :, j0:j0 + Ck[t], :] = cp.reshape(Ck[t], 128, 128).transpose(1, 0, 2)
            ids = up.reshape(Ck[t], 128).T            # [128, Ck]
            lin = ids.T.reshape(-1)
            w = lin.reshape(Ck[t] * 8, 16).T
            idx16[c, :, j0 * 8:(j0 + Ck[t]) * 8] = np.tile(w, (8, 1))
            xg[c, :, j0 * 128:(j0 + Ck[t]) * 128] = \
                xs[ids].reshape(128, Ck[t] * 128)
            j0 += Ck[t]
    rin = np.ones((NC_, 128, NT), np.float32)
    rout = np.ones((NC_, 128, NT), np.float32)
    for c in range(NC_):
        for t in range(NT):
            lo = c * SH + t * 128
            hi = min(lo + 128, (c + 1) * SH)
            rin[c, :hi - lo, t] = rsi[lo:hi]
            rout[c, :hi - lo, t] = rso[lo:hi]
    return Ck, nch, S.reshape(NC_, 128, nch * 128), idx16, xg, rin, rout


def _build(g_meta):
    nc = bacc.Bacc(None, target_bir_lowering=False, num_swdge_queues=4)
    ext = {}
    for g in range(3):
        nch = g_meta[g][1]
        ext[f"S{g}"] = nc.dram_tensor(f"S{g}", [128, nch * 128], f16, kind="ExternalInput")
        ext[f"xg{g}"] = nc.dram_tensor(f"xg{g}", [128, nch * 128], f16, kind="ExternalInput")
        ext[f"idx{g}"] = nc.dram_tensor(f"idx{g}", [128, nch * 8], i16, kind="ExternalInput")
        ext[f"rin{g}"] = nc.dram_tensor(f"rin{g}", [128, NT], f32, kind="ExternalInput")
        ext[f"rout{g}"] = nc.dram_tensor(f"rout{g}", [128, NT], f32, kind="ExternalInput")
    ext["W1"] = nc.dram_tensor("W1", [D_IN, D_H], f16, kind="ExternalInput")
    ext["b1"] = nc.dram_tensor("b1", [1, D_H], f16, kind="ExternalInput")
    for L in (2, 3):
        ext[f"W{L}p"] = nc.dram_tensor(f"W{L}p", [3 * 128, D_H], f16, kind="ExternalInput")
    for nm, shp in [("fW1", [D_H, 128]), ("fb1", [1, 128]), ("fW2", [128, 64]),
                    ("fb2", [1, 64]), ("fW3", [64, 1]), ("fb3", [1, 1])]:
        ext[nm] = nc.dram_tensor(nm, shp, f32, kind="ExternalInput")
    y_ext = nc.dram_tensor("y", [1, 1], f32, kind="ExternalOutput")

    ident_d = nc.inline_tensor(np.eye(128, dtype=np.float32), name="ident")
    ones16_d = nc.inline_tensor(np.ones((1, 128), np.float16), name="ones16")
    ones32_d = nc.inline_tensor(np.ones((1, 1), np.float32), name="ones32")

    with tile.TileContext(nc) as tc:
        with (
            tc.tile_pool(name="cst", bufs=1) as cst,
            tc.tile_pool(name="meta", bufs=1) as meta,
            tc.tile_pool(name="g", bufs=4) as gp,
            tc.tile_pool(name="s", bufs=2) as sp,
            tc.tile_pool(name="xp", bufs=2) as xp,
            tc.tile_pool(name="ps", bufs=2, space="PSUM") as pp,
            tc.tile_pool(name="ps2", bufs=2, space="PSUM") as pp2,
            tc.tile_pool(name="dram", bufs=1, space="DRAM") as dram,
        ):
            nc.gpsimd.load_library(library_config.mlp)

            ident_t = cst.tile([128, 128], f32)
            nc.sync.dma_start(ident_t[:], ident_d[:])
            ones16 = cst.tile([1, 128], f16)
            nc.sync.dma_start(ones16[:], ones16_d[:])
            ones32 = cst.tile([1, 1], f32)
            nc.sync.dma_start(ones32[:], ones32_d[:])

            w1t = cst.tile([128, D_H], f16, name="w1t")
            nc.sync.dma_start(w1t[:], ext["W1"][:])
            b1t = cst.tile([1, D_H], f16, name="b1t")
            nc.sync.dma_start(b1t[:], ext["b1"][:])
            W_t = {}
            for L in (2, 3):
                W_t[L] = []
                for j in range(3):
                    w = cst.tile([128, D_H], f16, name=f"w{L}_{j}")
                    k = 128 if j < 2 else 49
                    nc.sync.dma_start(w[0:k, :], ext[f"W{L}p"][j * 128:j * 128 + k, :])
                    W_t[L].append(w)
            fW1_t = []
            for j in range(3):
                k = 128 if j < 2 else 48
                w = cst.tile([128, 128], f32, name=f"fw1_{j}")
                nc.sync.dma_start(w[0:k, :], ext["fW1"][j * 128:j * 128 + k, :])
                fW1_t.append(w)
            fW2_t = cst.tile([128, 64], f32)
            nc.sync.dma_start(fW2_t[:], ext["fW2"][:])
            fW3_t = cst.tile([64, 1], f32)
            nc.sync.dma_start(fW3_t[:], ext["fW3"][:])
            fb_t = {}
            for nm, w in [("fb1", 128), ("fb2", 64), ("fb3", 1)]:
                b = cst.tile([1, w], f32, name=f"{nm}t")
                nc.sync.dma_start(b[:], ext[nm][:])
                fb_t[nm] = b

            hfA, hfB, sb1, sb2 = [], [], [], []
            for g in range(3):
                hfA.append(dram.tile([N, DPAD], f16, addr_space="Shared", name=f"hfA{g}"))
                hfB.append(dram.tile([N, DPAD], f16, addr_space="Shared", name=f"hfB{g}"))
                sb1.append(dram.tile([SH, DPAD], f16, name=f"sb1_{g}"))
                sb2.append(dram.tile([SH, DPAD], f16, name=f"sb2_{g}"))
            pool_in = dram.tile([128, 3], f32)
            pool_out = dram.tile([128, 3], f32, addr_space="Shared")
            vec_b = dram.tile([1, 128], f32)

            macc = cst.tile([128, D_H], f32)
            nc.vector.memset(macc[:], 0.0)

            idx_sb, rin_sb, rout_sb = [], [], []
            for g in range(3):
                nch = g_meta[g][1]
                ix = meta.tile([128, nch * 8], i16, name=f"ix{g}")
                nc.sync.dma_start(ix[:], ext[f"idx{g}"][:])
                idx_sb.append(ix)
                ri = meta.tile([128, NT], f32, name=f"ri{g}")
                nc.sync.dma_start(ri[:], ext[f"rin{g}"][:])
                rin_sb.append(ri)
                ro = meta.tile([128, NT], f32, name=f"ro{g}")
                nc.sync.dma_start(ro[:], ext[f"rout{g}"][:])
                rout_sb.append(ro)

            PG = 88            # chunks per S/xg piece (22KB/partition, 2 bufs)

            def run_layer(L, g, qoff):
                """One GraphConv layer for graph g (1-indexed layer L)."""
                Ck, nch = g_meta[g][0], g_meta[g][1]
                DL = D_IN if L == 1 else D_H
                src_tab = None if L == 1 else (hfA[g] if L == 2 else hfB[g])
                cur = {"pi": -1, "gi": -1}

                def fetch_piece(pi):
                    p0 = pi * PG
                    pc = min(PG, nch - p0)
                    st = sp.tile([128, PG * 128], f16, tag="st")
                    nc.scalar.dma_start(st[:, 0:pc * 128],
                                        ext[f"S{g}"][:, p0 * 128:(p0 + pc) * 128])
                    cur["pi"], cur["st"] = pi, st
                    if L == 1:
                        xt = xp.tile([128, PG * 128], f16, tag="xgt")
                        nc.sync.dma_start(xt[:, 0:pc * 128],
                                          ext[f"xg{g}"][:, p0 * 128:(p0 + pc) * 128])
                        cur["gt"], cur["gi"] = xt, pi

                def fetch_group(gi):
                    g0 = gi * GQ
                    gc = min(GQ, nch - g0)
                    gt = gp.tile([128, GQ, DPAD], f16, tag="gt")
                    nc.gpsimd.dma_gather(
                        gt[:, 0:gc, :], src_tab[:],
                        idx_sb[g][:, g0 * 8:(g0 + gc) * 8],
                        gc * 128, gc * 128, DPAD,
                        queue_num=(gi + qoff) % 4)
                    cur["gi"], cur["gt"] = gi, gt

                fetch_piece(0)
                if L > 1:
                    fetch_group(0)
                j0 = 0
                for t in range(NT):
                    rows = 128 if t < NT - 1 else SH - (NT - 1) * 128
                    psum = pp.tile([128, D_H], f32, tag="agg")
                    for jc in range(Ck[t]):
                        j = j0 + jc
                        if j // PG != cur["pi"]:
                            fetch_piece(j // PG)
                        st = cur["st"]
                        ssl = j - cur["pi"] * PG
                        if L == 1:
                            rhs = cur["gt"][:, ssl * 128:(ssl + 1) * 128]
                        else:
                            if j // GQ != cur["gi"]:
                                fetch_group(j // GQ)
                            rhs = cur["gt"][:, j - cur["gi"] * GQ, 0:D_H]
                        nc.tensor.matmul(psum[:, 0:DL],
                                         st[:, ssl * 128:(ssl + 1) * 128], rhs,
                                         start=(jc == 0), stop=(jc == Ck[t] - 1))
                    j0 += Ck[t]
                    psum2 = pp2.tile([128, D_H], f32, tag="wout")
                    if L == 1:
                        zsb = gp.tile([128, D_IN], f32, tag="zsb1")
                        nc.scalar.activation(zsb[:], psum[:, 0:D_IN], AF.Copy,
                                             scale=rin_sb[g][:, t:t + 1])
                        tp = pp.tile([128, 128], f32, tag="tp")
                        nc.tensor.transpose(tp[:], zsb[:], ident_t[:])
                        at = gp.tile([128, 128], f16, tag="at")
                        nc.vector.tensor_copy(at[:], tp[:])
                        nc.tensor.matmul(psum2[:], at[:], w1t[:], start=True, stop=False)
                        nc.tensor.matmul(psum2[:], ones16[:], b1t[:],
                                         start=False, stop=True)
                    else:
                        zsb = gp.tile([128, D_H + 1], f32, tag="zsb")
                        nc.scalar.activation(zsb[:, 0:D_H], psum[:, 0:D_H], AF.Copy,
                                             scale=rin_sb[g][:, t:t + 1])
                        nc.vector.memset(zsb[:, D_H:D_H + 1], 1.0)
                        for j in range(3):
                            k = 128 if j < 2 else 49
                            tp = pp.tile([128, 128], f32, tag="tp")
                            nc.tensor.transpose(tp[0:k, :],
                                                zsb[:, j * 128:j * 128 + k], ident_t[:])
                            at = gp.tile([128, 128], f16, tag="at")
                            nc.vector.tensor_copy(at[0:k, :], tp[0:k, :])
                            nc.tensor.matmul(psum2[:], at[0:k, :], W_t[L][j][0:k, :],
                                             start=(j == 0), stop=(j == 2))
                    if L < 3:
                        dst_sb = sb1[g] if L == 1 else sb2[g]
                        hsb = gp.tile([128, D_H], f16, tag="hsb")
                        nc.scalar.activation(hsb[:], psum2[:], AF.Relu,
                                             scale=rout_sb[g][:, t:t + 1])
                        nc.sync.dma_start(dst_sb[t * 128:t * 128 + rows, 0:D_H],
                                          hsb[0:rows, :])
                    else:
                        hsb3 = gp.tile([128, D_H], f32, tag="hsb3")
                        nc.scalar.activation(hsb3[:], psum2[:], AF.Relu)
                        nc.vector.tensor_tensor(macc[0:rows, :], macc[0:rows, :],
                                                hsb3[0:rows, :], mybir.AluOpType.max)
                if L < 3:
                    nc.gpsimd.collective_compute(
                        "AllGather", mybir.AluOpType.bypass, replica_groups=[core_ids],
                        ins=[(sb1[g] if L == 1 else sb2[g]).opt()],
                        outs=[(hfA[g] if L == 1 else hfB[g]).opt()])

            qoff = 0
            for L in (1, 2, 3):
                for g in range(3):
                    run_layer(L, g, qoff)
                    qoff += 1

            # max over partitions via transpose + reduce, AllReduce, MLP
            pool_sb = cst.tile([128, 3], f32)
            for j in range(3):
                k = 128 if j < 2 else 48
                tp = pp.tile([128, 128], f32, tag="tp")
                nc.tensor.transpose(tp[0:k, :], macc[:, j * 128:j * 128 + k], ident_t[:])
                nc.vector.tensor_reduce(pool_sb[0:k, j:j + 1], tp[0:k, :],
                                        mybir.AxisListType.X, mybir.AluOpType.max)
            nc.sync.dma_start(pool_in[:], pool_sb[:])
            nc.gpsimd.collective_compute(
                "AllReduce", mybir.AluOpType.max, replica_groups=[core_ids],
                ins=[pool_in.opt()], outs=[pool_out.opt()])
            pool_t = cst.tile([128, 3], f32)
            nc.sync.dma_start(pool_t[:], pool_out[:])

            z1p = pp2.tile([1, 128], f32, tag="z")
            for j in range(3):
                k = 128 if j < 2 else 48
                nc.tensor.matmul(z1p[:], pool_t[0:k, j:j + 1], fW1_t[j][0:k, :],
                                 start=(j == 0), stop=False)
            nc.tensor.matmul(z1p[:], ones32[:], fb_t["fb1"][:], start=False, stop=True)
            z1s = cst.tile([1, 128], f32)
            nc.scalar.activation(z1s[:], z1p[:], AF.Relu)
            nc.sync.dma_start(vec_b[:], z1s[:])
            z1T = cst.tile([128, 1], f32)
            nc.sync.dma_start(z1T[:], vec_b[0, :].rearrange("(p o) -> p o", o=1))
            z2p = pp2.tile([1, 64], f32, tag="z")
            nc.tensor.matmul(z2p[:], z1T[:], fW2_t[:], start=True, stop=False)
            nc.tensor.matmul(z2p[:], ones32[:], fb_t["fb2"][:], start=False, stop=True)
            z2s = cst.tile([1, 64], f32)
            nc.scalar.activation(z2s[:], z2p[:], AF.Relu)
            nc.sync.dma_start(vec_b[0:1, 0:64], z2s[:])
            z2T = cst.tile([64, 1], f32)
            nc.sync.dma_start(z2T[:], vec_b[0, 0:64].rearrange("(p o) -> p o", o=1))
            z3p = pp2.tile([1, 1], f32, tag="z")
            nc.tensor.matmul(z3p[:], z2T[:], fW3_t[:], start=True, stop=False)
            nc.tensor.matmul(z3p[:], ones32[:], fb_t["fb3"][:], start=False, stop=True)
            ys = cst.tile([1, 1], f32)
            nc.scalar.activation(ys[:], z3p[:], AF.Sigmoid)
            nc.sync.dma_start(y_ext[:], ys[:])

    nc.compile()
    return nc


def kernel(**inputs):
    g_meta = []
    for g, (s, d, xn) in enumerate([("src1", "dst1", "x1"), ("src2", "dst2", "x2"),
                                    ("src3", "dst3", "x3")]):
        g_meta.append(_prep_graph(inputs[s], inputs[d], inputs[xn]))
    nc = _build(g_meta)
    # fold biases into W2/W3's third row-block (row 48 = bias; at's row 48 = 1)
    Wp = {}
    for L in (2, 3):
        W = np.asarray(inputs[f"W{L}"], np.float32)
        b = np.asarray(inputs[f"b{L}"], np.float32).reshape(-1)
        blk = np.zeros((3 * 128, D_H), np.float16)
        blk[0:128] = W[0:128]
        blk[128:256] = W[128:256]
        blk[256:256 + 48] = W[256:304]
        blk[256 + 48] = b
        Wp[L] = blk
    in_maps = []
    for c in range(NC_):
        m = {}
        for g in range(3):
            Ck, nch, S, idx16, xg, rin, rout = g_meta[g]
            m[f"S{g}"] = S[c]
            m[f"xg{g}"] = xg[c]
            m[f"idx{g}"] = idx16[c]
            m[f"rin{g}"] = rin[c]
            m[f"rout{g}"] = rout[c]
        m["W1"] = np.asarray(inputs["W1"], np.float32).astype(np.float16)
        m["b1"] = np.asarray(inputs["b1"], np.float32).reshape(1, -1).astype(np.float16)
        m["W2p"] = Wp[2]
        m["W3p"] = Wp[3]
        m["fW1"] = np.asarray(inputs["fW1"], np.float32)
        m["fW2"] = np.asarray(inputs["fW2"], np.float32)
        m["fW3"] = np.asarray(inputs["fW3"], np.float32).reshape(64, 1)
        for nm in ["fb1", "fb2", "fb3"]:
            m[nm] = np.asarray(inputs[nm], np.float32).reshape(1, -1)
        in_maps.append(m)
    res = run_bass_kernel_spmd(nc, in_maps, core_ids)
    globals()["LAST"] = res
    return np.asarray(res.results[0]["y"], np.float32).reshape(1)


# revision 12
# speedup vs baseline: 57.7232x; 57.7232x over previous
"""3-branch GCN (DGL GraphConv x3 + max-pool + MLP head) on 8 TRN2 NeuronCores.

Sharding: destination nodes (2500/core). L1's x[src] gather is a static
permutation of the input, so it is pre-gathered (and rsqrt(outdeg)-prescaled)
on the host and streamed with dense DMAs. L2/L3 gather h[src] rows from a
replicated DRAM table with batched SWDGE dma_gathers (1024 rows each, round-
robined over the 4 SWDGE queues so all four DSP pairs generate descriptors
concurrently), aggregate via count-matrix fp16 matmuls into PSUM (per-tile
unique-src dedup folds edge multiplicity into S, built host-side and streamed
from DRAM), and apply the dense W matmul per dst tile. Layer outputs are
AllGathered; layers run graph-interleaved (layer-major) so each AllGather
hides under the other two graphs' compute. Max-pool is local + a final
AllReduce(max); the tiny MLP head runs replicated on every core.
"""
import numpy as np
import concourse.bass as bass
import concourse.bacc as bacc
import concourse.tile as tile
import concourse.mybir as mybir
from concourse import library_config
from concourse.bass_utils import run_bass_kernel_spmd

NC_ = 8
N = 20000
E = 320000
SH = N // NC_          # 2500 nodes per core
NT = 20                # dst tiles per core (19 full + 68-node partial)
D_IN, D_H = 128, 304
DPAD = 384             # fp16 row pad for 768B (256B-mult) rows
GQ = 8                 # chunks per dma_gather (1024 rows = SWDGE ring size)
f16, f32 = mybir.dt.float16, mybir.dt.float32
i16 = mybir.dt.int16
AF = mybir.ActivationFunctionType
core_ids = list(range(NC_))


def _prep_graph(src, dst, x):
    """Per-core deduped edge metadata, host-built S, pre-gathered L1 operand."""
    src = np.asarray(src).astype(np.int64)
    dst = np.asarray(dst).astype(np.int64)
    outdeg = np.bincount(src, minlength=N).clip(1).astype(np.float32)
    indeg = np.bincount(dst, minlength=N).clip(1).astype(np.float32)
    rso = (1.0 / np.sqrt(outdeg)).astype(np.float32)
    rsi = (1.0 / np.sqrt(indeg)).astype(np.float32)
    xs = (np.asarray(x, np.float32) * rso[:, None]).astype(np.float16)  # [N,128]
    # per (core, tile): unique srcs + count matrix columns
    uniqs = [[None] * NT for _ in range(NC_)]
    cnts = [[None] * NT for _ in range(NC_)]
    for c in range(NC_):
        m = (dst // SH) == c
        es, ed = src[m], dst[m] - c * SH
        for t in range(NT):
            tm = (ed // 128) == t
            u, inv = np.unique(es[tm], return_inverse=True)
            cm = np.zeros((max(len(u), 1), 128), np.float16)
            np.add.at(cm, (inv, ed[tm] - t * 128), 1.0)
            uniqs[c][t] = u if len(u) else np.zeros(1, np.int64)
            cnts[c][t] = cm
    Ck = [max(int(np.ceil(len(uniqs[c][t]) / 128)) for c in range(NC_)) or 1
          for t in range(NT)]
    nch = sum(Ck)
    S = np.zeros((NC_, 128, nch, 128), np.float16)   # [slot, chunk, dstrow]
    idx16 = np.zeros((NC_, 128, nch * 8), np.int16)
    xg = np.zeros((NC_, 128, nch * 128), np.float16)
    for c in range(NC_):
        j0 = 0
        for t in range(NT):
            u, cm = uniqs[c][t], cnts[c][t]
            npad = Ck[t] * 128
            up = np.zeros(npad, np.int64)
            up[:len(u)] = u
            cp = np.zeros((npad, 128), np.float16)
            cp[:len(u)] = cm
            # slot-major: slot i of chunk k = up[k*128 + i%... linear i = k*128+p
            S[c, :, j0:j0 + Ck[t], :] = cp.reshape(Ck[t], 128, 128).transpose(1, 0, 2)
            ids = up.reshape(Ck[t], 128).T            # [128, Ck]
            lin = ids.T.reshape(-1)
            w = lin.reshape(Ck[t] * 8, 16).T
            idx16[c, :, j0 * 8:(j0 + Ck[t]) * 8] = np.tile(w, (8, 1))
            xg[c, :, j0 * 128:(j0 + Ck[t]) * 128] = \
                xs[ids].reshape(128, Ck[t] * 128)
            j0 += Ck[t]
    rin = np.ones((NC_, 128, NT), np.float32)
    rout = np.ones((NC_, 128, NT), np.float32)
    for c in range(NC_):
        for t in range(NT):
            lo = c * SH + t * 128
            hi = min(lo + 128, (c + 1) * SH)
            rin[c, :hi - lo, t] = rsi[lo:hi]
            rout[c, :hi - lo, t] = rso[lo:hi]
    return Ck, nch, S.reshape(NC_, 128, nch * 128), idx16, xg, rin, rout


def _build(g_meta):
    nc = bacc.Bacc(None, target_bir_lowering=False, num_swdge_queues=4)
    ext = {}
    for g in range(3):
        nch = g_meta[g][1]
        ext[f"S{g}"] = nc.dram_tensor(f"S{g}", [128, nch * 128], f16, kind="ExternalInput")
        ext[f"xg{g}"] = nc.dram_tensor(f"xg{g}", [128, nch * 128], f16, kind="ExternalInput")
        ext[f"idx{g}"] = nc.dram_tensor(f"idx{g}", [128, nch * 8], i16, kind="ExternalInput")
        ext[f"rin{g}"] = nc.dram_tensor(f"rin{g}", [128, NT], f32, kind="ExternalInput")
        ext[f"rout{g}"] = nc.dram_tensor(f"rout{g}", [128, NT], f32, kind="ExternalInput")
    ext["W1"] = nc.dram_tensor("W1", [D_IN, D_H], f16, kind="ExternalInput")
    ext["b1"] = nc.dram_tensor("b1", [1, D_H], f16, kind="ExternalInput")
    for L in (2, 3):
        ext[f"W{L}p"] = nc.dram_tensor(f"W{L}p", [3 * 128, D_H], f16, kind="ExternalInput")
    for nm, shp in [("fW1", [D_H, 128]), ("fb1", [1, 128]), ("fW2", [128, 64]),
                    ("fb2", [1, 64]), ("fW3", [64, 1]), ("fb3", [1, 1])]:
        ext[nm] = nc.dram_tensor(nm, shp, f32, kind="ExternalInput")
    y_ext = nc.dram_tensor("y", [1, 1], f32, kind="ExternalOutput")

    ident_d = nc.inline_tensor(np.eye(128, dtype=np.float32), name="ident")
    ones16_d = nc.inline_tensor(np.ones((1, 128), np.float16), name="ones16")
    ones32_d = nc.inline_tensor(np.ones((1, 1), np.float32), name="ones32")

    with tile.TileContext(nc) as tc:
        with (
            tc.tile_pool(name="cst", bufs=1) as cst,
            tc.tile_pool(name="meta", bufs=1) as meta,
            tc.tile_pool(name="g", bufs=4) as gp,
            tc.tile_pool(name="s", bufs=2) as sp,
            tc.tile_pool(name="xp", bufs=2) as xp,
            tc.tile_pool(name="ps", bufs=2, space="PSUM") as pp,
            tc.tile_pool(name="ps2", bufs=2, space="PSUM") as pp2,
            tc.tile_pool(name="dram", bufs=1, space="DRAM") as dram,
        ):
            nc.gpsimd.load_library(library_config.mlp)

            ident_t = cst.tile([128, 128], f32)
            nc.sync.dma_start(ident_t[:], ident_d[:])
            ones16 = cst.tile([1, 128], f16)
            nc.sync.dma_start(ones16[:], ones16_d[:])
            ones32 = cst.tile([1, 1], f32)
            nc.sync.dma_start(ones32[:], ones32_d[:])

            w1t = cst.tile([128, D_H], f16, name="w1t")
            nc.sync.dma_start(w1t[:], ext["W1"][:])
            b1t = cst.tile([1, D_H], f16, name="b1t")
            nc.sync.dma_start(b1t[:], ext["b1"][:])
            W_t = {}
            for L in (2, 3):
                W_t[L] = []
                for j in range(3):
                    w = cst.tile([128, D_H], f16, name=f"w{L}_{j}")
                    k = 128 if j < 2 else 49
                    nc.sync.dma_start(w[0:k, :], ext[f"W{L}p"][j * 128:j * 128 + k, :])
                    W_t[L].append(w)
            fW1_t = []
            for j in range(3):
                k = 128 if j < 2 else 48
                w = cst.tile([128, 128], f32, name=f"fw1_{j}")
                nc.sync.dma_start(w[0:k, :], ext["fW1"][j * 128:j * 128 + k, :])
                fW1_t.append(w)
            fW2_t = cst.tile([128, 64], f32)
            nc.sync.dma_start(fW2_t[:], ext["fW2"][:])
            fW3_t = cst.tile([64, 1], f32)
            nc.sync.dma_start(fW3_t[:], ext["fW3"][:])
            fb_t = {}
            for nm, w in [("fb1", 128), ("fb2", 64), ("fb3", 1)]:
                b = cst.tile([1, w], f32, name=f"{nm}t")
                nc.sync.dma_start(b[:], ext[nm][:])
                fb_t[nm] = b

            hfA, hfB, sb1, sb2 = [], [], [], []
            for g in range(3):
                hfA.append(dram.tile([N, DPAD], f16, addr_space="Shared", name=f"hfA{g}"))
                hfB.append(dram.tile([N, DPAD], f16, addr_space="Shared", name=f"hfB{g}"))
                sb1.append(dram.tile([SH, DPAD], f16, name=f"sb1_{g}"))
                sb2.append(dram.tile([SH, DPAD], f16, name=f"sb2_{g}"))
            pool_in = dram.tile([128, 3], f32)
            pool_out = dram.tile([128, 3], f32, addr_space="Shared")
            vec_b = dram.tile([1, 128], f32)

            macc = cst.tile([128, D_H], f32)
            nc.vector.memset(macc[:], 0.0)

            idx_sb, rin_sb, rout_sb = [], [], []
            for g in range(3):
                nch = g_meta[g][1]
                ix = meta.tile([128, nch * 8], i16, name=f"ix{g}")
                nc.sync.dma_start(ix[:], ext[f"idx{g}"][:])
                idx_sb.append(ix)
                ri = meta.tile([128, NT], f32, name=f"ri{g}")
                nc.sync.dma_start(ri[:], ext[f"rin{g}"][:])
                rin_sb.append(ri)
                ro = meta.tile([128, NT], f32, name=f"ro{g}")
                nc.sync.dma_start(ro[:], ext[f"rout{g}"][:])
                rout_sb.append(ro)

            PG = 32            # chunks per S/xg piece (8KB/partition, 2 bufs)

            def run_layer(L, g, qoff):
                """One GraphConv layer for graph g (1-indexed layer L)."""
                Ck, nch = g_meta[g][0], g_meta[g][1]
                DL = D_IN if L == 1 else D_H
                src_tab = None if L == 1 else (hfA[g] if L == 2 else hfB[g])
                cur = {"pi": -1, "gi": -1}

                def fetch_piece(pi):
                    p0 = pi * PG
                    pc = min(PG, nch - p0)
                    st = sp.tile([128, PG * 128], f16, tag="st")
                    nc.scalar.dma_start(st[:, 0:pc * 128],
                                        ext[f"S{g}"][:, p0 * 128:(p0 + pc) * 128])
                    cur["pi"], cur["st"] = pi, st
                    if L == 1:
                        xt = xp.tile([128, PG * 128], f16, tag="xgt")
                        nc.sync.dma_start(xt[:, 0:pc * 128],
                                          ext[f"xg{g}"][:, p0 * 128:(p0 + pc) * 128])
                        cur["gt"], cur["gi"] = xt, pi

                def fetch_group(gi):
                    g0 = gi * GQ
                    gc = min(GQ, nch - g0)
                    gt = gp.tile([128, GQ, DPAD], f16, tag="gt")
                    nc.gpsimd.dma_gather(
                        gt[:, 0:gc, :], src_tab[:],
                        idx_sb[g][:, g0 * 8:(g0 + gc) * 8],
                        gc * 128, gc * 128, DPAD,
                        queue_num=(gi + qoff) % 4)
                    cur["gi"], cur["gt"] = gi, gt

                fetch_piece(0)
                if L > 1:
                    fetch_group(0)
                j0 = 0
                for t in range(NT):
                    rows = 128 if t < NT - 1 else SH - (NT - 1) * 128
                    psum = pp.tile([128, D_H], f32, tag="agg")
                    for jc in range(Ck[t]):
                        j = j0 + jc
                        if j // PG != cur["pi"]:
                            fetch_piece(j // PG)
                        st = cur["st"]
                        ssl = j - cur["pi"] * PG
                        if L == 1:
                            rhs = cur["gt"][:, ssl * 128:(ssl + 1) * 128]
                        else:
                            if j // GQ != cur["gi"]:
                                fetch_group(j // GQ)
                            rhs = cur["gt"][:, j - cur["gi"] * GQ, 0:D_H]
                        nc.tensor.matmul(psum[:, 0:DL],
                                         st[:, ssl * 128:(ssl + 1) * 128], rhs,
                                         start=(jc == 0), stop=(jc == Ck[t] - 1))
                    j0 += Ck[t]
                    psum2 = pp2.tile([128, D_H], f32, tag="wout")
                    if L == 1:
                        zsb = gp.tile([128, D_IN], f32, tag="zsb1")
                        nc.scalar.activation(zsb[:], psum[:, 0:D_IN], AF.Copy,
                                             scale=rin_sb[g][:, t:t + 1])
                        tp = pp.tile([128, 128], f32, tag="tp")
                        nc.tensor.transpose(tp[:], zsb[:], ident_t[:])
                        at = gp.tile([128, 128], f16, tag="at")
                        nc.vector.tensor_copy(at[:], tp[:])
                        nc.tensor.matmul(psum2[:], at[:], w1t[:], start=True, stop=False)
                        nc.tensor.matmul(psum2[:], ones16[:], b1t[:],
                                         start=False, stop=True)
                    else:
                        zsb = gp.tile([128, D_H + 1], f32, tag="zsb")
                        nc.scalar.activation(zsb[:, 0:D_H], psum[:, 0:D_H], AF.Copy,
                                             scale=rin_sb[g][:, t:t + 1])
                        nc.vector.memset(zsb[:, D_H:D_H + 1], 1.0)
                        for j in range(3):
                            k = 128 if j < 2 else 49
                            tp = pp.tile([128, 128], f32, tag="tp")
                            nc.tensor.transpose(tp[0:k, :],
                                                zsb[:, j * 128:j * 128 + k], ident_t[:])
                            at = gp.tile([128, 128], f16, tag="at")
                            nc.vector.tensor_copy(at[0:k, :], tp[0:k, :])
                            nc.tensor.matmul(psum2[:], at[0:k, :], W_t[L][j][0:k, :],
                                             start=(j == 0), stop=(j == 2))
                    if L < 3:
                        dst_sb = sb1[g] if L == 1 else sb2[g]
                        hsb = gp.tile([128, D_H], f16, tag="hsb")
                        nc.scalar.activation(hsb[:], psum2[:], AF.Relu,
                                             scale=rout_sb[g][:, t:t + 1])
                        nc.sync.dma_start(dst_sb[t * 128:t * 128 + rows, 0:D_H],
                                          hsb[0:rows, :])
                    else:
                        hsb3 = gp.tile([128, D_H], f32, tag="hsb3")
                        nc.scalar.activation(hsb3[:], psum2[:], AF.Relu)
                        nc.vector.tensor_tensor(macc[0:rows, :], macc[0:rows, :],
                                                hsb3[0:rows, :], mybir.AluOpType.max)
                if L < 3:
                    nc.gpsimd.collective_compute(
                        "AllGather", mybir.AluOpType.bypass, replica_groups=[core_ids],
                        ins=[(sb1[g] if L == 1 else sb2[g]).opt()],
                        outs=[(hfA[g] if L == 1 else hfB[g]).opt()])

            qoff = 0
            for L in (1, 2, 3):
                for g in range(3):
                    run_layer(L, g, qoff)
                    qoff += 1

            # max over partitions via transpose + reduce, AllReduce, MLP
            pool_sb = cst.tile([128, 3], f32)
            for j in range(3):
                k = 128 if j < 2 else 48
                tp = pp.tile([128, 128], f32, tag="tp")
                nc.tensor.transpose(tp[0:k, :], macc[:, j * 128:j * 128 + k], ident_t[:])
                nc.vector.tensor_reduce(pool_sb[0:k, j:j + 1], tp[0:k, :],
                                        mybir.AxisListType.X, mybir.AluOpType.max)
            nc.sync.dma_start(pool_in[:], pool_sb[:])
            nc.gpsimd.collective_compute(
                "AllReduce", mybir.AluOpType.max, replica_groups=[core_ids],
                ins=[pool_in.opt()], outs=[pool_out.opt()])
            pool_t = cst.tile([128, 3], f32)
            nc.sync.dma_start(pool_t[:], pool_out[:])

            z1p = pp2.tile([1, 128], f32, tag="z")
            for j in range(3):
                k = 128 if j < 2 else 48
                nc.tensor.matmul(z1p[:], pool_t[0:k, j:j + 1], fW1_t[j][0:k, :],
                                 start=(j == 0), stop=False)
            nc.tensor.matmul(z1p[:], ones32[:], fb_t["fb1"][:], start=False, stop=True)
            z1s = cst.tile([1, 128], f32)
            nc.scalar.activation(z1s[:], z1p[:], AF.Relu)
            nc.sync.dma_start(vec_b[:], z1s[:])
            z1T = cst.tile([128, 1], f32)
            nc.sync.dma_start(z1T[:], vec_b[0, :].rearrange("(p o) -> p o", o=1))
            z2p = pp2.tile([1, 64], f32, tag="z")
            nc.tensor.matmul(z2p[:], z1T[:], fW2_t[:], start=True, stop=False)
            nc.tensor.matmul(z2p[:], ones32[:], fb_t["fb2"][:], start=False, stop=True)
            z2s = cst.tile([1, 64], f32)
            nc.scalar.activation(z2s[:], z2p[:], AF.Relu)
            nc.sync.dma_start(vec_b[0:1, 0:64], z2s[:])
            z2T = cst.tile([64, 1], f32)
            nc.sync.dma_start(z2T[:], vec_b[0, 0:64].rearrange("(p o) -> p o", o=1))
            z3p = pp2.tile([1, 1], f32, tag="z")
            nc.tensor.matmul(z3p[:], z2T[:], fW3_t[:], start=True, stop=False)
            nc.tensor.matmul(z3p[:], ones32[:], fb_t["fb3"][:], start=False, stop=True)
            ys = cst.tile([1, 1], f32)
            nc.scalar.activation(ys[:], z3p[:], AF.Sigmoid)
            nc.sync.dma_start(y_ext[:], ys[:])

    nc.compile()
    return nc


def kernel(**inputs):
    g_meta = []
    for g, (s, d, xn) in enumerate([("src1", "dst1", "x1"), ("src2", "dst2", "x2"),
                                    ("src3", "dst3", "x3")]):
        g_meta.append(_prep_graph(inputs[s], inputs[d], inputs[xn]))
    nc = _build(g_meta)
    # fold biases into W2/W3's third row-block (row 48 = bias; at's row 48 = 1)
    Wp = {}
    for L in (2, 3):
        W = np.asarray(inputs[f"W{L}"], np.float32)
        b = np.asarray(inputs[f"b{L}"], np.float32).reshape(-1)
        blk = np.zeros((3 * 128, D_H), np.float16)
        blk[0:128] = W[0:128]
        blk[128:256] = W[128:256]
        blk[256:256 + 48] = W[256:304]
        blk[256 + 48] = b
        Wp[L] = blk
    in_maps = []
    for c in range(NC_):
        m = {}
        for g in range(3):
            Ck, nch, S, idx16, xg, rin, rout = g_meta[g]
            m[f"S{g}"] = S[c]
            m[f"xg{g}"] = xg[c]
            m[f"idx{g}"] = idx16[c]
            m[f"rin{g}"] = rin[c]
            m[f"rout{g}"] = rout[c]
        m["W1"] = np.asarray(inputs["W1"], np.float32).astype(np.float16)
        m["b1"] = np.asarray(inputs["b1"], np.float32).reshape(1, -1).astype(np.float16)
        m["W2p"] = Wp[2]
        m["W3p"] = Wp[3]
        m["fW1"] = np.asarray(inputs["fW1"], np.float32)
        m["fW2"] = np.asarray(inputs["fW2"], np.float32)
        m["fW3"] = np.asarray(inputs["fW3"], np.float32).reshape(64, 1)
        for nm in ["fb1", "fb2", "fb3"]:
            m[nm] = np.asarray(inputs[nm], np.float32).reshape(1, -1)
        in_maps.append(m)
    res = run_bass_kernel_spmd(nc, in_maps, core_ids)
    globals()["LAST"] = res
    return np.asarray(res.results[0]["y"], np.float32).reshape(1)


# revision 20
# speedup vs baseline: 66.0367x; 1.1440x over previous
"""3-branch GCN (DGL GraphConv x3 + max-pool + MLP head) on 8 TRN2 NeuronCores.

Sharding: destination nodes (2500/core). L1's x[src] gather is a static
permutation of the input, so it is pre-gathered (and rsqrt(outdeg)-prescaled)
on the host and streamed with dense DMAs. L2/L3 gather h[src] rows from a
replicated DRAM table with batched SWDGE dma_gathers (1024 rows each, round-
robined over the 4 SWDGE queues so all four DSP pairs generate descriptors
concurrently), aggregate via count-matrix fp16 matmuls into PSUM (per-tile
unique-src dedup folds edge multiplicity into S, built host-side and streamed
from DRAM), and apply the dense W matmul per dst tile. Layer outputs are
AllGathered; layers run graph-interleaved (layer-major) so each AllGather
hides under the other two graphs' compute. Max-pool is local + a final
AllReduce(max); the tiny MLP head runs replicated on every core.
"""
import numpy as np
import concourse.bass as bass
import concourse.bacc as bacc
import concourse.tile as tile
import concourse.mybir as mybir
from concourse import library_config
from concourse.bass_utils import run_bass_kernel_spmd

NC_ = 8
N = 20000
E = 320000
SH = N // NC_          # 2500 nodes per core
NT = 20                # dst tiles per core (19 full + 68-node partial)
D_IN, D_H = 128, 304
DPAD = 512             # fp8 row pad for 512B (256B-mult) rows
GQ = 8                 # chunks per dma_gather (1024 rows = SWDGE ring size)
f16, f32 = mybir.dt.float16, mybir.dt.float32
f8 = mybir.dt.float8e4
i16 = mybir.dt.int16
AF = mybir.ActivationFunctionType
core_ids = list(range(NC_))


def _prep_graph(src, dst, x):
    """Per-core deduped edge metadata, host-built S, pre-gathered L1 operand."""
    src = np.asarray(src).astype(np.int64)
    dst = np.asarray(dst).astype(np.int64)
    outdeg = np.bincount(src, minlength=N).clip(1).astype(np.float32)
    indeg = np.bincount(dst, minlength=N).clip(1).astype(np.float32)
    rso = (1.0 / np.sqrt(outdeg)).astype(np.float32)
    rsi = (1.0 / np.sqrt(indeg)).astype(np.float32)
    xs = (np.asarray(x, np.float32) * rso[:, None]).astype(np.float16)  # [N,128]
    # per (core, tile): unique srcs + count matrix columns
    uniqs = [[None] * NT for _ in range(NC_)]
    cnts = [[None] * NT for _ in range(NC_)]
    for c in range(NC_):
        m = (dst // SH) == c
        es, ed = src[m], dst[m] - c * SH
        for t in range(NT):
            tm = (ed // 128) == t
            u, inv = np.unique(es[tm], return_inverse=True)
            cm = np.zeros((max(len(u), 1), 128), np.float16)
            np.add.at(cm, (inv, ed[tm] - t * 128), 1.0)
            uniqs[c][t] = u if len(u) else np.zeros(1, np.int64)
            cnts[c][t] = cm
    # even chunk counts so fp8 DoubleRow chunk-pairs never straddle groups
    Ck = [max(2, 2 * int(np.ceil(max(int(np.ceil(len(uniqs[c][t]) / 128))
                                     for c in range(NC_)) / 2)))
          for t in range(NT)]
    nch = sum(Ck)
    S = np.zeros((NC_, 128, nch, 128), np.float16)   # [slot, chunk, dstrow]
    idx16 = np.zeros((NC_, 128, nch * 8), np.int16)
    xg = np.zeros((NC_, 128, nch * 128), np.float16)
    for c in range(NC_):
        j0 = 0
        for t in range(NT):
            u, cm = uniqs[c][t], cnts[c][t]
            npad = Ck[t] * 128
            up = np.zeros(npad, np.int64)
            up[:len(u)] = u
            cp = np.zeros((npad, 128), np.float16)
            cp[:len(u)] = cm
            # slot-major: slot i of chunk k = up[k*128 + i%... linear i = k*128+p
            S[c, :, j0:j0 + Ck[t], :] = cp.reshape(Ck[t], 128, 128).transpose(1, 0, 2)
            ids = up.reshape(Ck[t], 128).T            # [128, Ck]
            lin = ids.T.reshape(-1)
            w = lin.reshape(Ck[t] * 8, 16).T
            idx16[c, :, j0 * 8:(j0 + Ck[t]) * 8] = np.tile(w, (8, 1))
            xg[c, :, j0 * 128:(j0 + Ck[t]) * 128] = \
                xs[ids].reshape(128, Ck[t] * 128)
            j0 += Ck[t]
    rin = np.ones((NC_, 128, NT), np.float32)
    rout = np.ones((NC_, 128, NT), np.float32)
    for c in range(NC_):
        for t in range(NT):
            lo = c * SH + t * 128
            hi = min(lo + 128, (c + 1) * SH)
            rin[c, :hi - lo, t] = rsi[lo:hi]
            rout[c, :hi - lo, t] = rso[lo:hi]
    import ml_dtypes
    S = S.reshape(NC_, 128, nch * 128)
    S8 = S.astype(ml_dtypes.float8_e4m3)
    return Ck, nch, S, S8, idx16, xg, rin, rout


def _build(g_meta):
    nc = bacc.Bacc(None, target_bir_lowering=False, num_swdge_queues=4)
    ext = {}
    for g in range(3):
        nch = g_meta[g][1]
        ext[f"S{g}"] = nc.dram_tensor(f"S{g}", [128, nch * 128], f16, kind="ExternalInput")
        ext[f"S8{g}"] = nc.dram_tensor(f"S8{g}", [128, nch * 128], f8, kind="ExternalInput")
        ext[f"xg{g}"] = nc.dram_tensor(f"xg{g}", [128, nch * 128], f16, kind="ExternalInput")
        ext[f"idx{g}"] = nc.dram_tensor(f"idx{g}", [128, nch * 8], i16, kind="ExternalInput")
        ext[f"rin{g}"] = nc.dram_tensor(f"rin{g}", [128, NT], f32, kind="ExternalInput")
        ext[f"rout{g}"] = nc.dram_tensor(f"rout{g}", [128, NT], f32, kind="ExternalInput")
    ext["W1"] = nc.dram_tensor("W1", [D_IN, D_H], f16, kind="ExternalInput")
    ext["b1"] = nc.dram_tensor("b1", [1, D_H], f16, kind="ExternalInput")
    for L in (2, 3):
        ext[f"W{L}p"] = nc.dram_tensor(f"W{L}p", [3 * 128, D_H], f16, kind="ExternalInput")
    for nm, shp in [("fW1", [D_H, 128]), ("fb1", [1, 128]), ("fW2", [128, 64]),
                    ("fb2", [1, 64]), ("fW3", [64, 1]), ("fb3", [1, 1])]:
        ext[nm] = nc.dram_tensor(nm, shp, f32, kind="ExternalInput")
    y_ext = nc.dram_tensor("y", [1, 1], f32, kind="ExternalOutput")

    ident_d = nc.inline_tensor(np.eye(128, dtype=np.float32), name="ident")
    ones16_d = nc.inline_tensor(np.ones((1, 128), np.float16), name="ones16")
    ones32_d = nc.inline_tensor(np.ones((1, 1), np.float32), name="ones32")

    with tile.TileContext(nc) as tc:
        with (
            tc.tile_pool(name="cst", bufs=1) as cst,
            tc.tile_pool(name="meta", bufs=1) as meta,
            tc.tile_pool(name="g", bufs=4) as gp,
            tc.tile_pool(name="s", bufs=2) as sp,
            tc.tile_pool(name="xp", bufs=2) as xp,
            tc.tile_pool(name="ps", bufs=2, space="PSUM") as pp,
            tc.tile_pool(name="ps2", bufs=2, space="PSUM") as pp2,
            tc.tile_pool(name="dram", bufs=1, space="DRAM") as dram,
        ):
            nc.gpsimd.load_library(library_config.mlp)

            ident_t = cst.tile([128, 128], f32)
            nc.sync.dma_start(ident_t[:], ident_d[:])
            ones16 = cst.tile([1, 128], f16)
            nc.sync.dma_start(ones16[:], ones16_d[:])
            ones32 = cst.tile([1, 1], f32)
            nc.sync.dma_start(ones32[:], ones32_d[:])

            w1t = cst.tile([128, D_H], f16, name="w1t")
            nc.sync.dma_start(w1t[:], ext["W1"][:])
            b1t = cst.tile([1, D_H], f16, name="b1t")
            nc.sync.dma_start(b1t[:], ext["b1"][:])
            W_t = {}
            for L in (2, 3):
                W_t[L] = []
                for j in range(3):
                    w = cst.tile([128, D_H], f16, name=f"w{L}_{j}")
                    k = 128 if j < 2 else 49
                    nc.sync.dma_start(w[0:k, :], ext[f"W{L}p"][j * 128:j * 128 + k, :])
                    W_t[L].append(w)
            fW1_t = []
            for j in range(3):
                k = 128 if j < 2 else 48
                w = cst.tile([128, 128], f32, name=f"fw1_{j}")
                nc.sync.dma_start(w[0:k, :], ext["fW1"][j * 128:j * 128 + k, :])
                fW1_t.append(w)
            fW2_t = cst.tile([128, 64], f32)
            nc.sync.dma_start(fW2_t[:], ext["fW2"][:])
            fW3_t = cst.tile([64, 1], f32)
            nc.sync.dma_start(fW3_t[:], ext["fW3"][:])
            fb_t = {}
            for nm, w in [("fb1", 128), ("fb2", 64), ("fb3", 1)]:
                b = cst.tile([1, w], f32, name=f"{nm}t")
                nc.sync.dma_start(b[:], ext[nm][:])
                fb_t[nm] = b

            hfA, hfB, sb1, sb2 = [], [], [], []
            for g in range(3):
                hfA.append(dram.tile([N, DPAD], f8, addr_space="Shared", name=f"hfA{g}"))
                hfB.append(dram.tile([N, DPAD], f8, addr_space="Shared", name=f"hfB{g}"))
                sb1.append(dram.tile([SH, DPAD], f8, name=f"sb1_{g}"))
                sb2.append(dram.tile([SH, DPAD], f8, name=f"sb2_{g}"))
            pool_in = dram.tile([128, 3], f32)
            pool_out = dram.tile([128, 3], f32, addr_space="Shared")
            vec_b = dram.tile([1, 128], f32)

            macc = cst.tile([128, D_H], f32)
            nc.vector.memset(macc[:], 0.0)

            idx_sb, rin_sb, rout_sb = [], [], []
            for g in range(3):
                nch = g_meta[g][1]
                ix = meta.tile([128, nch * 8], i16, name=f"ix{g}")
                nc.sync.dma_start(ix[:], ext[f"idx{g}"][:])
                idx_sb.append(ix)
                ri = meta.tile([128, NT], f32, name=f"ri{g}")
                nc.sync.dma_start(ri[:], ext[f"rin{g}"][:])
                rin_sb.append(ri)
                ro = meta.tile([128, NT], f32, name=f"ro{g}")
                nc.sync.dma_start(ro[:], ext[f"rout{g}"][:])
                rout_sb.append(ro)

            PG = 32            # chunks per S/xg piece (8KB/partition, 2 bufs)

            def run_layer(L, g, qoff):
                """One GraphConv layer for graph g (1-indexed layer L)."""
                Ck, nch = g_meta[g][0], g_meta[g][1]
                DL = D_IN if L == 1 else D_H
                src_tab = None if L == 1 else (hfA[g] if L == 2 else hfB[g])
                cur = {"pi": -1, "gi": -1}

                def fetch_piece(pi):
                    p0 = pi * PG
                    pc = min(PG, nch - p0)
                    if L == 1:
                        st = sp.tile([128, PG, 128], f16, tag="st")
                        nc.scalar.dma_start(
                            st[:, 0:pc, :].rearrange("p c e -> p (c e)"),
                            ext[f"S{g}"][:, p0 * 128:(p0 + pc) * 128])
                        xt = xp.tile([128, PG * 128], f16, tag="xgt")
                        nc.sync.dma_start(xt[:, 0:pc * 128],
                                          ext[f"xg{g}"][:, p0 * 128:(p0 + pc) * 128])
                        cur["gt"], cur["gi"] = xt, pi
                    else:
                        st = sp.tile([128, PG, 128], f8, tag="st8")
                        nc.scalar.dma_start(
                            st[:, 0:pc, :].rearrange("p c e -> p (c e)"),
                            ext[f"S8{g}"][:, p0 * 128:(p0 + pc) * 128])
                    cur["pi"], cur["st"] = pi, st

                def fetch_group(gi):
                    g0 = gi * GQ
                    gc = min(GQ, nch - g0)
                    gt = gp.tile([128, GQ, DPAD], f8, tag="gt")
                    nc.gpsimd.dma_gather(
                        gt[:, 0:gc, :], src_tab[:],
                        idx_sb[g][:, g0 * 8:(g0 + gc) * 8],
                        gc * 128, gc * 128, DPAD,
                        queue_num=(gi + qoff) % 4)
                    cur["gi"], cur["gt"] = gi, gt

                fetch_piece(0)
                if L > 1:
                    fetch_group(0)
                j0 = 0
                for t in range(NT):
                    rows = 128 if t < NT - 1 else SH - (NT - 1) * 128
                    psum = pp.tile([128, D_H], f32, tag="agg")
                    step = 1 if L == 1 else 2
                    for jc in range(0, Ck[t], step):
                        j = j0 + jc
                        if j // PG != cur["pi"]:
                            fetch_piece(j // PG)
                        st = cur["st"]
                        ssl = j - cur["pi"] * PG
                        if L == 1:
                            rhs = cur["gt"][:, ssl * 128:(ssl + 1) * 128]
                            nc.tensor.matmul(psum[:, 0:DL], st[:, ssl, :], rhs,
                                             start=(jc == 0),
                                             stop=(jc == Ck[t] - 1))
                        else:
                            if j // GQ != cur["gi"]:
                                fetch_group(j // GQ)
                            sl = j - cur["gi"] * GQ
                            nc.tensor.matmul(
                                psum[:, 0:DL], st[:, ssl:ssl + 2, :],
                                cur["gt"][:, sl:sl + 2, 0:D_H],
                                start=(jc == 0), stop=(jc == Ck[t] - 2),
                                perf_mode=mybir.MatmulPerfMode.DoubleRow)
                    j0 += Ck[t]
                    psum2 = pp2.tile([128, D_H], f32, tag="wout")
                    if L == 1:
                        zsb = gp.tile([128, D_IN], f32, tag="zsb1")
                        nc.scalar.activation(zsb[:], psum[:, 0:D_IN], AF.Copy,
                                             scale=rin_sb[g][:, t:t + 1])
                        tp = pp.tile([128, 128], f32, tag="tp")
                        nc.tensor.transpose(tp[:], zsb[:], ident_t[:])
                        at = gp.tile([128, 128], f16, tag="at")
                        nc.vector.tensor_copy(at[:], tp[:])
                        nc.tensor.matmul(psum2[:], at[:], w1t[:], start=True, stop=False)
                        nc.tensor.matmul(psum2[:], ones16[:], b1t[:],
                                         start=False, stop=True)
                    else:
                        zsb = gp.tile([128, D_H + 1], f32, tag="zsb")
                        nc.scalar.activation(zsb[:, 0:D_H], psum[:, 0:D_H], AF.Copy,
                                             scale=rin_sb[g][:, t:t + 1])
                        nc.vector.memset(zsb[:, D_H:D_H + 1], 1.0)
                        for j in range(3):
                            k = 128 if j < 2 else 49
                            tp = pp.tile([128, 128], f32, tag="tp")
                            nc.tensor.transpose(tp[0:k, :],
                                                zsb[:, j * 128:j * 128 + k], ident_t[:])
                            at = gp.tile([128, 128], f16, tag="at")
                            nc.vector.tensor_copy(at[0:k, :], tp[0:k, :])
                            nc.tensor.matmul(psum2[:], at[0:k, :], W_t[L][j][0:k, :],
                                             start=(j == 0), stop=(j == 2))
                    if L < 3:
                        dst_sb = sb1[g] if L == 1 else sb2[g]
                        hsb = gp.tile([128, D_H], f8, tag="hsb")
                        nc.scalar.activation(hsb[:], psum2[:], AF.Relu,
                                             scale=rout_sb[g][:, t:t + 1])
                        nc.sync.dma_start(dst_sb[t * 128:t * 128 + rows, 0:D_H],
                                          hsb[0:rows, :])
                    else:
                        hsb3 = gp.tile([128, D_H], f32, tag="hsb3")
                        nc.scalar.activation(hsb3[:], psum2[:], AF.Relu)
                        nc.vector.tensor_tensor(macc[0:rows, :], macc[0:rows, :],
                                                hsb3[0:rows, :], mybir.AluOpType.max)
                if L < 3:
                    nc.gpsimd.collective_compute(
                        "AllGather", mybir.AluOpType.bypass, replica_groups=[core_ids],
                        ins=[(sb1[g] if L == 1 else sb2[g]).opt()],
                        outs=[(hfA[g] if L == 1 else hfB[g]).opt()])

            qoff = 0
            for L in (1, 2, 3):
                for g in range(3):
                    run_layer(L, g, qoff)
                    qoff += 1

            # max over partitions via transpose + reduce, AllReduce, MLP
            pool_sb = cst.tile([128, 3], f32)
            for j in range(3):
                k = 128 if j < 2 else 48
                tp = pp.tile([128, 128], f32, tag="tp")
                nc.tensor.transpose(tp[0:k, :], macc[:, j * 128:j * 128 + k], ident_t[:])
                nc.vector.tensor_reduce(pool_sb[0:k, j:j + 1], tp[0:k, :],
                                        mybir.AxisListType.X, mybir.AluOpType.max)
            nc.sync.dma_start(pool_in[:], pool_sb[:])
            nc.gpsimd.collective_compute(
                "AllReduce", mybir.AluOpType.max, replica_groups=[core_ids],
                ins=[pool_in.opt()], outs=[pool_out.opt()])
            pool_t = cst.tile([128, 3], f32)
            nc.sync.dma_start(pool_t[:], pool_out[:])

            z1p = pp2.tile([1, 128], f32, tag="z")
            for j in range(3):
                k = 128 if j < 2 else 48
                nc.tensor.matmul(z1p[:], pool_t[0:k, j:j + 1], fW1_t[j][0:k, :],
                                 start=(j == 0), stop=False)
            nc.tensor.matmul(z1p[:], ones32[:], fb_t["fb1"][:], start=False, stop=True)
            z1s = cst.tile([1, 128], f32)
            nc.scalar.activation(z1s[:], z1p[:], AF.Relu)
            nc.sync.dma_start(vec_b[:], z1s[:])
            z1T = cst.tile([128, 1], f32)
            nc.sync.dma_start(z1T[:], vec_b[0, :].rearrange("(p o) -> p o", o=1))
            z2p = pp2.tile([1, 64], f32, tag="z")
            nc.tensor.matmul(z2p[:], z1T[:], fW2_t[:], start=True, stop=False)
            nc.tensor.matmul(z2p[:], ones32[:], fb_t["fb2"][:], start=False, stop=True)
            z2s = cst.tile([1, 64], f32)
            nc.scalar.activation(z2s[:], z2p[:], AF.Relu)
            nc.sync.dma_start(vec_b[0:1, 0:64], z2s[:])
            z2T = cst.tile([64, 1], f32)
            nc.sync.dma_start(z2T[:], vec_b[0, 0:64].rearrange("(p o) -> p o", o=1))
            z3p = pp2.tile([1, 1], f32, tag="z")
            nc.tensor.matmul(z3p[:], z2T[:], fW3_t[:], start=True, stop=False)
            nc.tensor.matmul(z3p[:], ones32[:], fb_t["fb3"][:], start=False, stop=True)
            ys = cst.tile([1, 1], f32)
            nc.scalar.activation(ys[:], z3p[:], AF.Sigmoid)
            nc.sync.dma_start(y_ext[:], ys[:])

    nc.compile()
    return nc


def kernel(**inputs):
    g_meta = []
    for g, (s, d, xn) in enumerate([("src1", "dst1", "x1"), ("src2", "dst2", "x2"),
                                    ("src3", "dst3", "x3")]):
        g_meta.append(_prep_graph(inputs[s], inputs[d], inputs[xn]))
    nc = _build(g_meta)
    # fold biases into W2/W3's third row-block (row 48 = bias; at's row 48 = 1)
    Wp = {}
    for L in (2, 3):
        W = np.asarray(inputs[f"W{L}"], np.float32)
        b = np.asarray(inputs[f"b{L}"], np.float32).reshape(-1)
        blk = np.zeros((3 * 128, D_H), np.float16)
        blk[0:128] = W[0:128]
        blk[128:256] = W[128:256]
        blk[256:256 + 48] = W[256:304]
        blk[256 + 48] = b
        Wp[L] = blk
    in_maps = []
    for c in range(NC_):
        m = {}
        for g in range(3):
            Ck, nch, S, S8, idx16, xg, rin, rout = g_meta[g]
            m[f"S{g}"] = S[c]
            m[f"S8{g}"] = S8[c]
            m[f"xg{g}"] = xg[c]
            m[f"idx{g}"] = idx16[c]
            m[f"rin{g}"] = rin[c]
            m[f"rout{g}"] = rout[c]
        m["W1"] = np.asarray(inputs["W1"], np.float32).astype(np.float16)
        m["b1"] = np.asarray(inputs["b1"], np.float32).reshape(1, -1).astype(np.float16)
        m["W2p"] = Wp[2]
        m["W3p"] = Wp[3]
        m["fW1"] = np.asarray(inputs["fW1"], np.float32)
        m["fW2"] = np.asarray(inputs["fW2"], np.float32)
        m["fW3"] = np.asarray(inputs["fW3"], np.float32).reshape(64, 1)
        for nm in ["fb1", "fb2", "fb3"]:
            m[nm] = np.asarray(inputs[nm], np.float32).reshape(1, -1)
        in_maps.append(m)
    res = run_bass_kernel_spmd(nc, in_maps, core_ids)
    globals()["LAST"] = res
    return np.asarray(res.results[0]["y"], np.float32).reshape(1)


# revision 25
# speedup vs baseline: 69.6551x; 1.0548x over previous
"""3-branch GCN (DGL GraphConv x3 + max-pool + MLP head) on 8 TRN2 NeuronCores.

Sharding: destination nodes (2500/core). L1's x[src] gather is a static
permutation of the input, so it is pre-gathered (and rsqrt(outdeg)-prescaled)
on the host and streamed with dense DMAs. L2/L3 gather h[src] rows from a
replicated DRAM table with batched SWDGE dma_gathers (1024 rows each, round-
robined over the 4 SWDGE queues so all four DSP pairs generate descriptors
concurrently), aggregate via count-matrix fp16 matmuls into PSUM (per-tile
unique-src dedup folds edge multiplicity into S, built host-side and streamed
from DRAM), and apply the dense W matmul per dst tile. Layer outputs are
AllGathered; layers run graph-interleaved (layer-major) so each AllGather
hides under the other two graphs' compute. Max-pool is local + a final
AllReduce(max); the tiny MLP head runs replicated on every core.
"""
import numpy as np
import concourse.bass as bass
import concourse.bacc as bacc
import concourse.tile as tile
import concourse.mybir as mybir
from concourse import library_config
from concourse.bass_utils import run_bass_kernel_spmd

NC_ = 8
N = 20000
E = 320000
SH = N // NC_          # 2500 nodes per core
NT = 20                # dst tiles per core (19 full + 68-node partial)
D_IN, D_H = 128, 304
DPAD = 512             # fp8 row pad for 512B (256B-mult) rows
GQ = 8                 # chunks per dma_gather (1024 rows = SWDGE ring size)
f16, f32 = mybir.dt.float16, mybir.dt.float32
f8 = mybir.dt.float8e4
i16 = mybir.dt.int16
AF = mybir.ActivationFunctionType
core_ids = list(range(NC_))


def _prep_graph(src, dst, x):
    """Per-core deduped edge metadata, host-built S, pre-gathered L1 operand."""
    src = np.asarray(src).astype(np.int64)
    dst = np.asarray(dst).astype(np.int64)
    outdeg = np.bincount(src, minlength=N).clip(1).astype(np.float32)
    indeg = np.bincount(dst, minlength=N).clip(1).astype(np.float32)
    rso = (1.0 / np.sqrt(outdeg)).astype(np.float32)
    rsi = (1.0 / np.sqrt(indeg)).astype(np.float32)
    xs = (np.asarray(x, np.float32) * rso[:, None]).astype(np.float16)  # [N,128]
    # per (core, tile): unique srcs + count matrix columns
    uniqs = [[None] * NT for _ in range(NC_)]
    cnts = [[None] * NT for _ in range(NC_)]
    for c in range(NC_):
        m = (dst // SH) == c
        es, ed = src[m], dst[m] - c * SH
        for t in range(NT):
            tm = (ed // 128) == t
            u, inv = np.unique(es[tm], return_inverse=True)
            cm = np.zeros((max(len(u), 1), 128), np.float16)
            np.add.at(cm, (inv, ed[tm] - t * 128), 1.0)
            uniqs[c][t] = u if len(u) else np.zeros(1, np.int64)
            cnts[c][t] = cm
    # even chunk counts so fp8 DoubleRow chunk-pairs never straddle groups
    Ck = [max(2, 2 * int(np.ceil(max(int(np.ceil(len(uniqs[c][t]) / 128))
                                     for c in range(NC_)) / 2)))
          for t in range(NT)]
    nch = sum(Ck)
    S = np.zeros((NC_, 128, nch, 128), np.float16)   # [slot, chunk, dstrow]
    idx16 = np.zeros((NC_, 128, nch * 8), np.int16)
    xg = np.zeros((NC_, 128, nch * 128), np.float16)
    for c in range(NC_):
        j0 = 0
        for t in range(NT):
            u, cm = uniqs[c][t], cnts[c][t]
            npad = Ck[t] * 128
            up = np.zeros(npad, np.int64)
            up[:len(u)] = u
            cp = np.zeros((npad, 128), np.float16)
            cp[:len(u)] = cm
            # slot-major: slot i of chunk k = up[k*128 + i%... linear i = k*128+p
            S[c, :, j0:j0 + Ck[t], :] = cp.reshape(Ck[t], 128, 128).transpose(1, 0, 2)
            ids = up.reshape(Ck[t], 128).T            # [128, Ck]
            lin = ids.T.reshape(-1)
            w = lin.reshape(Ck[t] * 8, 16).T
            idx16[c, :, j0 * 8:(j0 + Ck[t]) * 8] = np.tile(w, (8, 1))
            xg[c, :, j0 * 128:(j0 + Ck[t]) * 128] = \
                xs[ids].reshape(128, Ck[t] * 128)
            j0 += Ck[t]
    rin = np.ones((NC_, 128, NT), np.float32)
    rout = np.ones((NC_, 128, NT), np.float32)
    for c in range(NC_):
        for t in range(NT):
            lo = c * SH + t * 128
            hi = min(lo + 128, (c + 1) * SH)
            rin[c, :hi - lo, t] = rsi[lo:hi]
            rout[c, :hi - lo, t] = rso[lo:hi]
    import ml_dtypes
    S8 = S.reshape(NC_, 128, nch * 128).astype(ml_dtypes.float8_e4m3)
    xg8 = xg.astype(ml_dtypes.float8_e4m3)
    return Ck, nch, S8, idx16, xg8, rin, rout


def _build(g_meta):
    nc = bacc.Bacc(None, target_bir_lowering=False, num_swdge_queues=4)
    ext = {}
    for g in range(3):
        nch = g_meta[g][1]
        ext[f"S8{g}"] = nc.dram_tensor(f"S8{g}", [128, nch * 128], f8, kind="ExternalInput")
        ext[f"xg{g}"] = nc.dram_tensor(f"xg{g}", [128, nch * 128], f8, kind="ExternalInput")
        ext[f"idx{g}"] = nc.dram_tensor(f"idx{g}", [128, nch * 8], i16, kind="ExternalInput")
        ext[f"rin{g}"] = nc.dram_tensor(f"rin{g}", [128, NT], f32, kind="ExternalInput")
        ext[f"rout{g}"] = nc.dram_tensor(f"rout{g}", [128, NT], f32, kind="ExternalInput")
    ext["W1"] = nc.dram_tensor("W1", [D_IN, D_H], f16, kind="ExternalInput")
    ext["b1"] = nc.dram_tensor("b1", [1, D_H], f16, kind="ExternalInput")
    for L in (2, 3):
        ext[f"W{L}p"] = nc.dram_tensor(f"W{L}p", [3 * 128, D_H], f16, kind="ExternalInput")
    for nm, shp in [("fW1", [D_H, 128]), ("fb1", [1, 128]), ("fW2", [128, 64]),
                    ("fb2", [1, 64]), ("fW3", [64, 1]), ("fb3", [1, 1])]:
        ext[nm] = nc.dram_tensor(nm, shp, f32, kind="ExternalInput")
    y_ext = nc.dram_tensor("y", [1, 1], f32, kind="ExternalOutput")

    ident_d = nc.inline_tensor(np.eye(128, dtype=np.float32), name="ident")
    ones16_d = nc.inline_tensor(np.ones((1, 128), np.float16), name="ones16")
    ones32_d = nc.inline_tensor(np.ones((1, 1), np.float32), name="ones32")

    with tile.TileContext(nc) as tc:
        with (
            tc.tile_pool(name="cst", bufs=1) as cst,
            tc.tile_pool(name="meta", bufs=1) as meta,
            tc.tile_pool(name="g", bufs=4) as gp,
            tc.tile_pool(name="gt", bufs=10) as gtp,
            tc.tile_pool(name="s", bufs=2) as sp,
            tc.tile_pool(name="xp", bufs=2) as xp,
            tc.tile_pool(name="ps", bufs=2, space="PSUM") as pp,
            tc.tile_pool(name="ps2", bufs=2, space="PSUM") as pp2,
            tc.tile_pool(name="dram", bufs=1, space="DRAM") as dram,
        ):
            nc.gpsimd.load_library(library_config.mlp)

            ident_t = cst.tile([128, 128], f32)
            nc.sync.dma_start(ident_t[:], ident_d[:])
            ones16 = cst.tile([1, 128], f16)
            nc.sync.dma_start(ones16[:], ones16_d[:])
            ones32 = cst.tile([1, 1], f32)
            nc.sync.dma_start(ones32[:], ones32_d[:])

            w1t = cst.tile([128, D_H], f16, name="w1t")
            nc.sync.dma_start(w1t[:], ext["W1"][:])
            b1t = cst.tile([1, D_H], f16, name="b1t")
            nc.sync.dma_start(b1t[:], ext["b1"][:])
            W_t = {}
            for L in (2, 3):
                W_t[L] = []
                for j in range(3):
                    w = cst.tile([128, D_H], f16, name=f"w{L}_{j}")
                    k = 128 if j < 2 else 49
                    nc.sync.dma_start(w[0:k, :], ext[f"W{L}p"][j * 128:j * 128 + k, :])
                    W_t[L].append(w)
            fW1_t = []
            for j in range(3):
                k = 128 if j < 2 else 48
                w = cst.tile([128, 128], f32, name=f"fw1_{j}")
                nc.sync.dma_start(w[0:k, :], ext["fW1"][j * 128:j * 128 + k, :])
                fW1_t.append(w)
            fW2_t = cst.tile([128, 64], f32)
            nc.sync.dma_start(fW2_t[:], ext["fW2"][:])
            fW3_t = cst.tile([64, 1], f32)
            nc.sync.dma_start(fW3_t[:], ext["fW3"][:])
            fb_t = {}
            for nm, w in [("fb1", 128), ("fb2", 64), ("fb3", 1)]:
                b = cst.tile([1, w], f32, name=f"{nm}t")
                nc.sync.dma_start(b[:], ext[nm][:])
                fb_t[nm] = b

            hfA, hfB, sb1, sb2 = [], [], [], []
            for g in range(3):
                hfA.append(dram.tile([N, DPAD], f8, addr_space="Shared", name=f"hfA{g}"))
                hfB.append(dram.tile([N, DPAD], f8, addr_space="Shared", name=f"hfB{g}"))
                sb1.append(dram.tile([SH, DPAD], f8, name=f"sb1_{g}"))
                sb2.append(dram.tile([SH, DPAD], f8, name=f"sb2_{g}"))
            pool_in = dram.tile([128, 3], f32)
            pool_out = dram.tile([128, 3], f32, addr_space="Shared")
            vec_b = dram.tile([1, 128], f32)

            macc = cst.tile([128, D_H], f32)
            nc.vector.memset(macc[:], 0.0)

            idx_sb, rin_sb, rout_sb = [], [], []
            for g in range(3):
                nch = g_meta[g][1]
                ix = meta.tile([128, nch * 8], i16, name=f"ix{g}")
                nc.sync.dma_start(ix[:], ext[f"idx{g}"][:])
                idx_sb.append(ix)
                ri = meta.tile([128, NT], f32, name=f"ri{g}")
                nc.sync.dma_start(ri[:], ext[f"rin{g}"][:])
                rin_sb.append(ri)
                ro = meta.tile([128, NT], f32, name=f"ro{g}")
                nc.sync.dma_start(ro[:], ext[f"rout{g}"][:])
                rout_sb.append(ro)

            PG = 32            # chunks per S/xg piece (8KB/partition, 2 bufs)

            def run_layer(L, g, qoff):
                """One GraphConv layer for graph g (1-indexed layer L)."""
                Ck, nch = g_meta[g][0], g_meta[g][1]
                DL = D_IN if L == 1 else D_H
                src_tab = None if L == 1 else (hfA[g] if L == 2 else hfB[g])
                cur = {"pi": -1, "gi": -1}

                def fetch_piece(pi):
                    p0 = pi * PG
                    pc = min(PG, nch - p0)
                    st = sp.tile([128, PG, 128], f8, tag="st8")
                    nc.scalar.dma_start(
                        st[:, 0:pc, :].rearrange("p c e -> p (c e)"),
                        ext[f"S8{g}"][:, p0 * 128:(p0 + pc) * 128])
                    if L == 1:
                        xt = xp.tile([128, PG, 128], f8, tag="xgt")
                        nc.sync.dma_start(
                            xt[:, 0:pc, :].rearrange("p c e -> p (c e)"),
                            ext[f"xg{g}"][:, p0 * 128:(p0 + pc) * 128])
                        cur["gt"], cur["gi"] = xt, pi
                    cur["pi"], cur["st"] = pi, st

                def fetch_group(gi):
                    g0 = gi * GQ
                    gc = min(GQ, nch - g0)
                    gt = gtp.tile([128, GQ, DPAD], f8, tag="gt")
                    nc.gpsimd.dma_gather(
                        gt[:, 0:gc, :], src_tab[:],
                        idx_sb[g][:, g0 * 8:(g0 + gc) * 8],
                        gc * 128, gc * 128, DPAD,
                        queue_num=(gi + qoff) % 4)
                    cur["gi"], cur["gt"] = gi, gt

                fetch_piece(0)
                if L > 1:
                    fetch_group(0)
                j0 = 0
                for t in range(NT):
                    rows = 128 if t < NT - 1 else SH - (NT - 1) * 128
                    psum = pp.tile([128, D_H], f32, tag="agg")
                    for jc in range(0, Ck[t], 2):
                        j = j0 + jc
                        if j // PG != cur["pi"]:
                            fetch_piece(j // PG)
                        st = cur["st"]
                        ssl = j - cur["pi"] * PG
                        if L == 1:
                            rhs = cur["gt"][:, ssl:ssl + 2, :]
                        else:
                            if j // GQ != cur["gi"]:
                                fetch_group(j // GQ)
                            sl = j - cur["gi"] * GQ
                            rhs = cur["gt"][:, sl:sl + 2, 0:D_H]
                        nc.tensor.matmul(
                            psum[:, 0:DL], st[:, ssl:ssl + 2, :], rhs,
                            start=(jc == 0), stop=(jc == Ck[t] - 2),
                            perf_mode=mybir.MatmulPerfMode.DoubleRow)
                    j0 += Ck[t]
                    psum2 = pp2.tile([128, D_H], f32, tag="wout")
                    if L == 1:
                        zsb = gp.tile([128, D_IN], f32, tag="zsb1")
                        nc.scalar.activation(zsb[:], psum[:, 0:D_IN], AF.Copy,
                                             scale=rin_sb[g][:, t:t + 1])
                        tp = pp.tile([128, 128], f32, tag="tp")
                        nc.tensor.transpose(tp[:], zsb[:], ident_t[:])
                        at = gp.tile([128, 128], f16, tag="at")
                        nc.vector.tensor_copy(at[:], tp[:])
                        nc.tensor.matmul(psum2[:], at[:], w1t[:], start=True, stop=False)
                        nc.tensor.matmul(psum2[:], ones16[:], b1t[:],
                                         start=False, stop=True)
                    else:
                        zsb = gp.tile([128, D_H + 1], f32, tag="zsb")
                        nc.scalar.activation(zsb[:, 0:D_H], psum[:, 0:D_H], AF.Copy,
                                             scale=rin_sb[g][:, t:t + 1])
                        nc.vector.memset(zsb[:, D_H:D_H + 1], 1.0)
                        for j in range(3):
                            k = 128 if j < 2 else 49
                            tp = pp.tile([128, 128], f32, tag="tp")
                            nc.tensor.transpose(tp[0:k, :],
                                                zsb[:, j * 128:j * 128 + k], ident_t[:])
                            at = gp.tile([128, 128], f16, tag="at")
                            nc.vector.tensor_copy(at[0:k, :], tp[0:k, :])
                            nc.tensor.matmul(psum2[:], at[0:k, :], W_t[L][j][0:k, :],
                                             start=(j == 0), stop=(j == 2))
                    if L < 3:
                        dst_sb = sb1[g] if L == 1 else sb2[g]
                        hsb = gp.tile([128, D_H], f8, tag="hsb")
                        nc.scalar.activation(hsb[:], psum2[:], AF.Relu,
                                             scale=rout_sb[g][:, t:t + 1])
                        nc.sync.dma_start(dst_sb[t * 128:t * 128 + rows, 0:D_H],
                                          hsb[0:rows, :])
                    else:
                        hsb3 = gp.tile([128, D_H], f32, tag="hsb3")
                        nc.scalar.activation(hsb3[:], psum2[:], AF.Relu)
                        nc.vector.tensor_tensor(macc[0:rows, :], macc[0:rows, :],
                                                hsb3[0:rows, :], mybir.AluOpType.max)
                if L < 3:
                    nc.gpsimd.collective_compute(
                        "AllGather", mybir.AluOpType.bypass, replica_groups=[core_ids],
                        ins=[(sb1[g] if L == 1 else sb2[g]).opt()],
                        outs=[(hfA[g] if L == 1 else hfB[g]).opt()])

            stages = [(1, 0), (1, 1), (2, 0), (1, 2), (2, 1),
                      (3, 0), (2, 2), (3, 1), (3, 2)]
            for qoff, (L, g) in enumerate(stages):
                run_layer(L, g, qoff)

            # max over partitions via transpose + reduce, AllReduce, MLP
            pool_sb = cst.tile([128, 3], f32)
            for j in range(3):
                k = 128 if j < 2 else 48
                tp = pp.tile([128, 128], f32, tag="tp")
                nc.tensor.transpose(tp[0:k, :], macc[:, j * 128:j * 128 + k], ident_t[:])
                nc.vector.tensor_reduce(pool_sb[0:k, j:j + 1], tp[0:k, :],
                                        mybir.AxisListType.X, mybir.AluOpType.max)
            nc.sync.dma_start(pool_in[:], pool_sb[:])
            nc.gpsimd.collective_compute(
                "AllReduce", mybir.AluOpType.max, replica_groups=[core_ids],
                ins=[pool_in.opt()], outs=[pool_out.opt()])
            pool_t = cst.tile([128, 3], f32)
            nc.sync.dma_start(pool_t[:], pool_out[:])

            z1p = pp2.tile([1, 128], f32, tag="z")
            for j in range(3):
                k = 128 if j < 2 else 48
                nc.tensor.matmul(z1p[:], pool_t[0:k, j:j + 1], fW1_t[j][0:k, :],
                                 start=(j == 0), stop=False)
            nc.tensor.matmul(z1p[:], ones32[:], fb_t["fb1"][:], start=False, stop=True)
            z1s = cst.tile([1, 128], f32)
            nc.scalar.activation(z1s[:], z1p[:], AF.Relu)
            nc.sync.dma_start(vec_b[:], z1s[:])
            z1T = cst.tile([128, 1], f32)
            nc.sync.dma_start(z1T[:], vec_b[0, :].rearrange("(p o) -> p o", o=1))
            z2p = pp2.tile([1, 64], f32, tag="z")
            nc.tensor.matmul(z2p[:], z1T[:], fW2_t[:], start=True, stop=False)
            nc.tensor.matmul(z2p[:], ones32[:], fb_t["fb2"][:], start=False, stop=True)
            z2s = cst.tile([1, 64], f32)
            nc.scalar.activation(z2s[:], z2p[:], AF.Relu)
            nc.sync.dma_start(vec_b[0:1, 0:64], z2s[:])
            z2T = cst.tile([64, 1], f32)
            nc.sync.dma_start(z2T[:], vec_b[0, 0:64].rearrange("(p o) -> p o", o=1))
            z3p = pp2.tile([1, 1], f32, tag="z")
            nc.tensor.matmul(z3p[:], z2T[:], fW3_t[:], start=True, stop=False)
            nc.tensor.matmul(z3p[:], ones32[:], fb_t["fb3"][:], start=False, stop=True)
            ys = cst.tile([1, 1], f32)
            nc.scalar.activation(ys[:], z3p[:], AF.Sigmoid)
            nc.sync.dma_start(y_ext[:], ys[:])

    nc.compile()
    return nc


def kernel(**inputs):
    g_meta = []
    for g, (s, d, xn) in enumerate([("src1", "dst1", "x1"), ("src2", "dst2", "x2"),
                                    ("src3", "dst3", "x3")]):
        g_meta.append(_prep_graph(inputs[s], inputs[d], inputs[xn]))
    nc = _build(g_meta)
    # fold biases into W2/W3's third row-block (row 48 = bias; at's row 48 = 1)
    Wp = {}
    for L in (2, 3):
        W = np.asarray(inputs[f"W{L}"], np.float32)
        b = np.asarray(inputs[f"b{L}"], np.float32).reshape(-1)
        blk = np.zeros((3 * 128, D_H), np.float16)
        blk[0:128] = W[0:128]
        blk[128:256] = W[128:256]
        blk[256:256 + 48] = W[256:304]
        blk[256 + 48] = b
        Wp[L] = blk
    in_maps = []
    for c in range(NC_):
        m = {}
        for g in range(3):
            Ck, nch, S8, idx16, xg8, rin, rout = g_meta[g]
            m[f"S8{g}"] = S8[c]
            m[f"xg{g}"] = xg8[c]
            m[f"idx{g}"] = idx16[c]
            m[f"rin{g}"] = rin[c]
            m[f"rout{g}"] = rout[c]
        m["W1"] = np.asarray(inputs["W1"], np.float32).astype(np.float16)
        m["b1"] = np.asarray(inputs["b1"], np.float32).reshape(1, -1).astype(np.float16)
        m["W2p"] = Wp[2]
        m["W3p"] = Wp[3]
        m["fW1"] = np.asarray(inputs["fW1"], np.float32)
        m["fW2"] = np.asarray(inputs["fW2"], np.float32)
        m["fW3"] = np.asarray(inputs["fW3"], np.float32).reshape(64, 1)
        for nm in ["fb1", "fb2", "fb3"]:
            m[nm] = np.asarray(inputs[nm], np.float32).reshape(1, -1)
        in_maps.append(m)
    res = run_bass_kernel_spmd(nc, in_maps, core_ids)
    globals()["LAST"] = res
    return np.asarray(res.results[0]["y"], np.float32).reshape(1)


# revision 30
# speedup vs baseline: 73.1601x; 1.0503x over previous
"""3-branch GCN (DGL GraphConv x3 + max-pool + MLP head) on 8 TRN2 NeuronCores.

Sharding: destination nodes (2500/core). L1's x[src] gather is a static
permutation of the input, so it is pre-gathered (and rsqrt(outdeg)-prescaled)
on the host and streamed with dense DMAs. L2/L3 gather h[src] rows from a
replicated DRAM table with batched SWDGE dma_gathers (1024 rows each, round-
robined over the 4 SWDGE queues so all four DSP pairs generate descriptors
concurrently), aggregate via count-matrix fp16 matmuls into PSUM (per-tile
unique-src dedup folds edge multiplicity into S, built host-side and streamed
from DRAM), and apply the dense W matmul per dst tile. Layer outputs are
AllGathered; layers run graph-interleaved (layer-major) so each AllGather
hides under the other two graphs' compute. Max-pool is local + a final
AllReduce(max); the tiny MLP head runs replicated on every core.
"""
import numpy as np
import concourse.bass as bass
import concourse.bacc as bacc
import concourse.tile as tile
import concourse.mybir as mybir
from concourse import library_config
from concourse.bass_utils import run_bass_kernel_spmd

NC_ = 8
N = 20000
E = 320000
SH = N // NC_          # 2500 nodes per core
NT = 20                # dst tiles per core (19 full + 68-node partial)
D_IN, D_H = 128, 304
DPAD = 512             # fp8 row pad for 512B (256B-mult) rows
GQ = 8                 # chunks per dma_gather (1024 rows = SWDGE ring size)
f16, f32 = mybir.dt.float16, mybir.dt.float32
f8 = mybir.dt.float8e4
i16 = mybir.dt.int16
AF = mybir.ActivationFunctionType
core_ids = list(range(NC_))


def _prep_graph(src, dst, x):
    """Per-core deduped edge metadata, host-built S, pre-gathered L1 operand."""
    src = np.asarray(src).astype(np.int64)
    dst = np.asarray(dst).astype(np.int64)
    outdeg = np.bincount(src, minlength=N).clip(1).astype(np.float32)
    indeg = np.bincount(dst, minlength=N).clip(1).astype(np.float32)
    rso = (1.0 / np.sqrt(outdeg)).astype(np.float32)
    rsi = (1.0 / np.sqrt(indeg)).astype(np.float32)
    xs = (np.asarray(x, np.float32) * rso[:, None]).astype(np.float16)  # [N,128]
    # per (core, tile): unique srcs + count matrix columns
    uniqs = [[None] * NT for _ in range(NC_)]
    cnts = [[None] * NT for _ in range(NC_)]
    for c in range(NC_):
        m = (dst // SH) == c
        es, ed = src[m], dst[m] - c * SH
        for t in range(NT):
            tm = (ed // 128) == t
            u, inv = np.unique(es[tm], return_inverse=True)
            cm = np.zeros((max(len(u), 1), 128), np.float16)
            np.add.at(cm, (inv, ed[tm] - t * 128), 1.0)
            uniqs[c][t] = u if len(u) else np.zeros(1, np.int64)
            cnts[c][t] = cm
    # even chunk counts so fp8 DoubleRow chunk-pairs never straddle groups
    Ck = [max(2, 2 * int(np.ceil(max(int(np.ceil(len(uniqs[c][t]) / 128))
                                     for c in range(NC_)) / 2)))
          for t in range(NT)]
    nch = sum(Ck)
    S = np.zeros((NC_, 128, nch, 128), np.float16)   # [slot, chunk, dstrow]
    idx16 = np.zeros((NC_, 128, nch * 8), np.int16)
    xg = np.zeros((NC_, 128, nch * 128), np.float16)
    for c in range(NC_):
        j0 = 0
        for t in range(NT):
            u, cm = uniqs[c][t], cnts[c][t]
            npad = Ck[t] * 128
            up = np.zeros(npad, np.int64)
            up[:len(u)] = u
            cp = np.zeros((npad, 128), np.float16)
            cp[:len(u)] = cm
            # slot-major: slot i of chunk k = up[k*128 + i%... linear i = k*128+p
            S[c, :, j0:j0 + Ck[t], :] = cp.reshape(Ck[t], 128, 128).transpose(1, 0, 2)
            ids = up.reshape(Ck[t], 128).T            # [128, Ck]
            lin = ids.T.reshape(-1)
            w = lin.reshape(Ck[t] * 8, 16).T
            idx16[c, :, j0 * 8:(j0 + Ck[t]) * 8] = np.tile(w, (8, 1))
            xg[c, :, j0 * 128:(j0 + Ck[t]) * 128] = \
                xs[ids].reshape(128, Ck[t] * 128)
            j0 += Ck[t]
    rin = np.ones((NC_, 128, NT), np.float32)
    rout = np.ones((NC_, 128, NT), np.float32)
    for c in range(NC_):
        for t in range(NT):
            lo = c * SH + t * 128
            hi = min(lo + 128, (c + 1) * SH)
            rin[c, :hi - lo, t] = rsi[lo:hi]
            rout[c, :hi - lo, t] = rso[lo:hi]
    import ml_dtypes
    S8 = S.reshape(NC_, 128, nch * 128).astype(ml_dtypes.float8_e4m3)
    xg8 = xg.astype(ml_dtypes.float8_e4m3)
    return Ck, nch, S8, idx16, xg8, rin, rout


def _build(g_meta):
    nc = bacc.Bacc(None, target_bir_lowering=False, num_swdge_queues=4)
    ext = {}
    for g in range(3):
        nch = g_meta[g][1]
        ext[f"S8{g}"] = nc.dram_tensor(f"S8{g}", [128, nch * 128], f8, kind="ExternalInput")
        ext[f"xg{g}"] = nc.dram_tensor(f"xg{g}", [128, nch * 128], f8, kind="ExternalInput")
        ext[f"idx{g}"] = nc.dram_tensor(f"idx{g}", [128, nch * 8], i16, kind="ExternalInput")
        ext[f"rin{g}"] = nc.dram_tensor(f"rin{g}", [128, NT], f32, kind="ExternalInput")
        ext[f"rout{g}"] = nc.dram_tensor(f"rout{g}", [128, NT], f32, kind="ExternalInput")
    ext["W1"] = nc.dram_tensor("W1", [D_IN, D_H], f16, kind="ExternalInput")
    ext["b1"] = nc.dram_tensor("b1", [1, D_H], f16, kind="ExternalInput")
    for L in (2, 3):
        ext[f"W{L}p"] = nc.dram_tensor(f"W{L}p", [3 * 128, D_H], f16, kind="ExternalInput")
    for nm, shp in [("fW1", [D_H, 128]), ("fb1", [1, 128]), ("fW2", [128, 64]),
                    ("fb2", [1, 64]), ("fW3", [64, 1]), ("fb3", [1, 1])]:
        ext[nm] = nc.dram_tensor(nm, shp, f32, kind="ExternalInput")
    y_ext = nc.dram_tensor("y", [1, 1], f32, kind="ExternalOutput")

    ident_d = nc.inline_tensor(np.eye(128, dtype=np.float32), name="ident")
    ones16_d = nc.inline_tensor(np.ones((1, 128), np.float16), name="ones16")
    ones32_d = nc.inline_tensor(np.ones((1, 1), np.float32), name="ones32")

    with tile.TileContext(nc) as tc:
        with (
            tc.tile_pool(name="cst", bufs=1) as cst,
            tc.tile_pool(name="meta", bufs=1) as meta,
            tc.tile_pool(name="g", bufs=4) as gp,
            tc.tile_pool(name="gt", bufs=10) as gtp,
            tc.tile_pool(name="s", bufs=2) as sp,
            tc.tile_pool(name="xp", bufs=2) as xp,
            tc.tile_pool(name="ps", bufs=3, space="PSUM") as pp,
            tc.tile_pool(name="pt", bufs=2, space="PSUM") as ppt,
            tc.tile_pool(name="ps2", bufs=3, space="PSUM") as pp2,
            tc.tile_pool(name="dram", bufs=1, space="DRAM") as dram,
        ):
            nc.gpsimd.load_library(library_config.mlp)

            ident_t = cst.tile([128, 128], f32)
            nc.sync.dma_start(ident_t[:], ident_d[:])
            ones16 = cst.tile([1, 128], f16)
            nc.sync.dma_start(ones16[:], ones16_d[:])
            ones32 = cst.tile([1, 1], f32)
            nc.sync.dma_start(ones32[:], ones32_d[:])

            w1t = cst.tile([128, D_H], f16, name="w1t")
            nc.sync.dma_start(w1t[:], ext["W1"][:])
            b1t = cst.tile([1, D_H], f16, name="b1t")
            nc.sync.dma_start(b1t[:], ext["b1"][:])
            W_t = {}
            for L in (2, 3):
                W_t[L] = []
                for j in range(3):
                    w = cst.tile([128, D_H], f16, name=f"w{L}_{j}")
                    k = 128 if j < 2 else 49
                    nc.sync.dma_start(w[0:k, :], ext[f"W{L}p"][j * 128:j * 128 + k, :])
                    W_t[L].append(w)
            fW1_t = []
            for j in range(3):
                k = 128 if j < 2 else 48
                w = cst.tile([128, 128], f32, name=f"fw1_{j}")
                nc.sync.dma_start(w[0:k, :], ext["fW1"][j * 128:j * 128 + k, :])
                fW1_t.append(w)
            fW2_t = cst.tile([128, 64], f32)
            nc.sync.dma_start(fW2_t[:], ext["fW2"][:])
            fW3_t = cst.tile([64, 1], f32)
            nc.sync.dma_start(fW3_t[:], ext["fW3"][:])
            fb_t = {}
            for nm, w in [("fb1", 128), ("fb2", 64), ("fb3", 1)]:
                b = cst.tile([1, w], f32, name=f"{nm}t")
                nc.sync.dma_start(b[:], ext[nm][:])
                fb_t[nm] = b

            hfA, hfB, sb1, sb2 = [], [], [], []
            for g in range(3):
                hfA.append(dram.tile([N, DPAD], f8, addr_space="Shared", name=f"hfA{g}"))
                hfB.append(dram.tile([N, DPAD], f8, addr_space="Shared", name=f"hfB{g}"))
                sb1.append(dram.tile([SH, DPAD], f8, name=f"sb1_{g}"))
                sb2.append(dram.tile([SH, DPAD], f8, name=f"sb2_{g}"))
            pool_in = dram.tile([128, 3], f32)
            pool_out = dram.tile([128, 3], f32, addr_space="Shared")
            vec_b = dram.tile([1, 128], f32)

            macc = cst.tile([128, D_H], f32)
            nc.vector.memset(macc[:], 0.0)

            idx_sb, rin_sb, rout_sb = [], [], []
            for g in range(3):
                nch = g_meta[g][1]
                ix = meta.tile([128, nch * 8], i16, name=f"ix{g}")
                nc.sync.dma_start(ix[:], ext[f"idx{g}"][:])
                idx_sb.append(ix)
                ri = meta.tile([128, NT], f32, name=f"ri{g}")
                nc.sync.dma_start(ri[:], ext[f"rin{g}"][:])
                rin_sb.append(ri)
                ro = meta.tile([128, NT], f32, name=f"ro{g}")
                nc.sync.dma_start(ro[:], ext[f"rout{g}"][:])
                rout_sb.append(ro)

            PG = 32            # chunks per S/xg piece (8KB/partition, 2 bufs)

            def run_layer(L, g, qoff):
                """One GraphConv layer for graph g (1-indexed layer L)."""
                Ck, nch = g_meta[g][0], g_meta[g][1]
                DL = D_IN if L == 1 else D_H
                src_tab = None if L == 1 else (hfA[g] if L == 2 else hfB[g])
                cur = {"pi": -1, "gi": -1}

                def fetch_piece(pi):
                    p0 = pi * PG
                    pc = min(PG, nch - p0)
                    st = sp.tile([128, PG, 128], f8, tag="st8")
                    nc.sync.dma_start(
                        st[:, 0:pc, :].rearrange("p c e -> p (c e)"),
                        ext[f"S8{g}"][:, p0 * 128:(p0 + pc) * 128])
                    if L == 1:
                        xt = xp.tile([128, PG, 128], f8, tag="xgt")
                        nc.sync.dma_start(
                            xt[:, 0:pc, :].rearrange("p c e -> p (c e)"),
                            ext[f"xg{g}"][:, p0 * 128:(p0 + pc) * 128])
                        cur["gt"], cur["gi"] = xt, pi
                    cur["pi"], cur["st"] = pi, st

                def fetch_group(gi):
                    g0 = gi * GQ
                    gc = min(GQ, nch - g0)
                    gt = gtp.tile([128, GQ, DPAD], f8, tag="gt")
                    nc.gpsimd.dma_gather(
                        gt[:, 0:gc, :], src_tab[:],
                        idx_sb[g][:, g0 * 8:(g0 + gc) * 8],
                        gc * 128, gc * 128, DPAD,
                        queue_num=(gi + qoff) % 4)
                    cur["gi"], cur["gt"] = gi, gt

                fetch_piece(0)
                if L > 1:
                    fetch_group(0)
                j0 = 0
                for t in range(NT):
                    rows = 128 if t < NT - 1 else SH - (NT - 1) * 128
                    psum = pp.tile([128, D_H], f32, tag="agg")
                    for jc in range(0, Ck[t], 2):
                        j = j0 + jc
                        if j // PG != cur["pi"]:
                            fetch_piece(j // PG)
                        st = cur["st"]
                        ssl = j - cur["pi"] * PG
                        if L == 1:
                            rhs = cur["gt"][:, ssl:ssl + 2, :]
                        else:
                            if j // GQ != cur["gi"]:
                                fetch_group(j // GQ)
                            sl = j - cur["gi"] * GQ
                            rhs = cur["gt"][:, sl:sl + 2, 0:D_H]
                        nc.tensor.matmul(
                            psum[:, 0:DL], st[:, ssl:ssl + 2, :], rhs,
                            start=(jc == 0), stop=(jc == Ck[t] - 2),
                            perf_mode=mybir.MatmulPerfMode.DoubleRow)
                    j0 += Ck[t]
                    psum2 = pp2.tile([128, D_H], f32, tag="wout")
                    if L == 1:
                        zsb = gp.tile([128, D_IN], f32, tag="zsb1")
                        nc.scalar.activation(zsb[:], psum[:, 0:D_IN], AF.Copy,
                                             scale=rin_sb[g][:, t:t + 1])
                        tp = ppt.tile([128, 128], f32, tag="tp")
                        nc.tensor.transpose(tp[:], zsb[:], ident_t[:])
                        at = gp.tile([128, 128], f16, tag="at")
                        nc.vector.tensor_copy(at[:], tp[:])
                        nc.tensor.matmul(psum2[:], at[:], w1t[:], start=True, stop=False)
                        nc.tensor.matmul(psum2[:], ones16[:], b1t[:],
                                         start=False, stop=True)
                    else:
                        zsb = gp.tile([128, D_H + 1], f32, tag="zsb")
                        nc.scalar.activation(zsb[:, 0:D_H], psum[:, 0:D_H], AF.Copy,
                                             scale=rin_sb[g][:, t:t + 1])
                        nc.vector.memset(zsb[:, D_H:D_H + 1], 1.0)
                        for j in range(3):
                            k = 128 if j < 2 else 49
                            tp = ppt.tile([128, 128], f32, tag="tp")
                            nc.tensor.transpose(tp[0:k, :],
                                                zsb[:, j * 128:j * 128 + k], ident_t[:])
                            at = gp.tile([128, 128], f16, tag="at")
                            nc.vector.tensor_copy(at[0:k, :], tp[0:k, :])
                            nc.tensor.matmul(psum2[:], at[0:k, :], W_t[L][j][0:k, :],
                                             start=(j == 0), stop=(j == 2))
                    if L < 3:
                        dst_sb = sb1[g] if L == 1 else sb2[g]
                        hsb = gp.tile([128, D_H], f8, tag="hsb")
                        nc.scalar.activation(hsb[:], psum2[:], AF.Relu,
                                             scale=rout_sb[g][:, t:t + 1])
                        nc.sync.dma_start(dst_sb[t * 128:t * 128 + rows, 0:D_H],
                                          hsb[0:rows, :])
                    else:
                        hsb3 = gp.tile([128, D_H], f32, tag="hsb3")
                        nc.scalar.activation(hsb3[:], psum2[:], AF.Relu)
                        nc.vector.tensor_tensor(macc[0:rows, :], macc[0:rows, :],
                                                hsb3[0:rows, :], mybir.AluOpType.max)
                if L < 3:
                    nc.gpsimd.collective_compute(
                        "AllGather", mybir.AluOpType.bypass, replica_groups=[core_ids],
                        ins=[(sb1[g] if L == 1 else sb2[g]).opt()],
                        outs=[(hfA[g] if L == 1 else hfB[g]).opt()])

            stages = [(1, 0), (1, 1), (2, 0), (1, 2), (2, 1),
                      (3, 0), (2, 2), (3, 1), (3, 2)]
            for qoff, (L, g) in enumerate(stages):
                run_layer(L, g, qoff)

            # max over partitions via transpose + reduce, AllReduce, MLP
            pool_sb = cst.tile([128, 3], f32)
            for j in range(3):
                k = 128 if j < 2 else 48
                tp = ppt.tile([128, 128], f32, tag="tp")
                nc.tensor.transpose(tp[0:k, :], macc[:, j * 128:j * 128 + k], ident_t[:])
                nc.vector.tensor_reduce(pool_sb[0:k, j:j + 1], tp[0:k, :],
                                        mybir.AxisListType.X, mybir.AluOpType.max)
            nc.sync.dma_start(pool_in[:], pool_sb[:])
            nc.gpsimd.collective_compute(
                "AllReduce", mybir.AluOpType.max, replica_groups=[core_ids],
                ins=[pool_in.opt()], outs=[pool_out.opt()])
            pool_t = cst.tile([128, 3], f32)
            nc.sync.dma_start(pool_t[:], pool_out[:])

            z1f = pp2.tile([128, D_H], f32, tag="wout")
            z1p = z1f[0:1, 0:128]
            for j in range(3):
                k = 128 if j < 2 else 48
                nc.tensor.matmul(z1p[:], pool_t[0:k, j:j + 1], fW1_t[j][0:k, :],
                                 start=(j == 0), stop=False)
            nc.tensor.matmul(z1p[:], ones32[:], fb_t["fb1"][:], start=False, stop=True)
            z1s = cst.tile([1, 128], f32)
            nc.scalar.activation(z1s[:], z1p[:], AF.Relu)
            nc.sync.dma_start(vec_b[:], z1s[:])
            z1T = cst.tile([128, 1], f32)
            nc.sync.dma_start(z1T[:], vec_b[0, :].rearrange("(p o) -> p o", o=1))
            z2f = pp2.tile([128, D_H], f32, tag="wout")
            z2p = z2f[0:1, 0:64]
            nc.tensor.matmul(z2p[:], z1T[:], fW2_t[:], start=True, stop=False)
            nc.tensor.matmul(z2p[:], ones32[:], fb_t["fb2"][:], start=False, stop=True)
            z2s = cst.tile([1, 64], f32)
            nc.scalar.activation(z2s[:], z2p[:], AF.Relu)
            nc.sync.dma_start(vec_b[0:1, 0:64], z2s[:])
            z2T = cst.tile([64, 1], f32)
            nc.sync.dma_start(z2T[:], vec_b[0, 0:64].rearrange("(p o) -> p o", o=1))
            z3f = pp2.tile([128, D_H], f32, tag="wout")
            z3p = z3f[0:1, 0:1]
            nc.tensor.matmul(z3p[:], z2T[:], fW3_t[:], start=True, stop=False)
            nc.tensor.matmul(z3p[:], ones32[:], fb_t["fb3"][:], start=False, stop=True)
            ys = cst.tile([1, 1], f32)
            nc.scalar.activation(ys[:], z3p[:], AF.Sigmoid)
            nc.sync.dma_start(y_ext[:], ys[:])

    nc.compile()
    return nc


def kernel(**inputs):
    g_meta = []
    for g, (s, d, xn) in enumerate([("src1", "dst1", "x1"), ("src2", "dst2", "x2"),
                                    ("src3", "dst3", "x3")]):
        g_meta.append(_prep_graph(inputs[s], inputs[d], inputs[xn]))
    nc = _build(g_meta)
    # fold biases into W2/W3's third row-block (row 48 = bias; at's row 48 = 1)
    Wp = {}
    for L in (2, 3):
        W = np.asarray(inputs[f"W{L}"], np.float32)
        b = np.asarray(inputs[f"b{L}"], np.float32).reshape(-1)
        blk = np.zeros((3 * 128, D_H), np.float16)
        blk[0:128] = W[0:128]
        blk[128:256] = W[128:256]
        blk[256:256 + 48] = W[256:304]
        blk[256 + 48] = b
        Wp[L] = blk
    in_maps = []
    for c in range(NC_):
        m = {}
        for g in range(3):
            Ck, nch, S8, idx16, xg8, rin, rout = g_meta[g]
            m[f"S8{g}"] = S8[c]
            m[f"xg{g}"] = xg8[c]
            m[f"idx{g}"] = idx16[c]
            m[f"rin{g}"] = rin[c]
            m[f"rout{g}"] = rout[c]
        m["W1"] = np.asarray(inputs["W1"], np.float32).astype(np.float16)
        m["b1"] = np.asarray(inputs["b1"], np.float32).reshape(1, -1).astype(np.float16)
        m["W2p"] = Wp[2]
        m["W3p"] = Wp[3]
        m["fW1"] = np.asarray(inputs["fW1"], np.float32)
        m["fW2"] = np.asarray(inputs["fW2"], np.float32)
        m["fW3"] = np.asarray(inputs["fW3"], np.float32).reshape(64, 1)
        for nm in ["fb1", "fb2", "fb3"]:
            m[nm] = np.asarray(inputs[nm], np.float32).reshape(1, -1)
        in_maps.append(m)
    res = run_bass_kernel_spmd(nc, in_maps, core_ids)
    globals()["LAST"] = res
    return np.asarray(res.results[0]["y"], np.float32).reshape(1)
